# revision 15
# baseline (speedup 1.0000x reference)
"""GATv2Conv batched-graph kernel for Trainium2 (8 NeuronCores, data-parallel).

Problem: B=16384 independent 9-node graphs, C_in=C_out=256, fixed edge list
(16 directed tree edges + 9 self-loops = 25 edges), GATv2 attention.

Per core (B/8 = 2048 graphs), per block of G=512 graphs:
  - host pre-transposes x to channel-major node-major layout
    xT[c, blk*9*G + s*G + g] (bf16) so all matmuls/vector slices are
    contiguous.
  - PE: channel-major xlT/xrT projections; graph-major xl_gm projection
    (lhsT = xT slices, rhs = W) for the aggregation; per-edge score
    matmuls att . leaky(s_e) accumulated into one [25, G] PSUM tile via
    att (x) onehot(e) stationaries; a subset of the per-edge adds as
    identity-stationary accumulating matmuls (s_e = I.T@xl_s + I.T@xr_d
    in PSUM); denominator segment-sum via a [25, 25] indicator matmul;
    small transposes for per-graph alpha.
  - DVE/GPSIMD: remaining per-edge adds (bf16 TT), leaky-relu as one
    fused scalar_tensor_tensor op max(0.2*s, s), and the alpha-weighted
    aggregation as non-RMW scalar_tensor_tensor chains in graph-major
    layout (alpha is a per-partition scalar there).
  - ACT: exp, PSUM->SBUF projection copies, some aggregation-chain heads
    (copy with scale).
  - Software pipelining: block b's projections are emitted interleaved
    with block b-1's softmax tail + aggregation so the PE never drains
    (keeps the HAM clock gate warm) and the vector engines always have
    ready work.
  - softmax needs no max-subtraction (scores are O(1)); bias is handled
    host-side (it is zeros in this problem).
"""

import sys

if "/opt/trn_rl_repo" not in sys.path:
    sys.path.insert(0, "/opt/trn_rl_repo")

import numpy as np
import ml_dtypes

import concourse.bass as bass
import concourse.bacc as bacc
import concourse.mybir as mybir
from concourse import tile
from concourse.bass_utils import run_bass_kernel_spmd

F32 = mybir.dt.float32
BF16 = mybir.dt.bfloat16

N_CORES = 8
B_TOTAL = 16384
BC = B_TOTAL // N_CORES          # graphs per core
NN = 9                           # nodes per graph
C = 256                          # channels
G = 512                          # graphs per block
NBLK = BC // G                   # blocks per core
NT = G // 128                    # 128-graph subtiles per block
NGT = NN * G                     # columns per (chunk, block)

# ---- static edge list, grouped by destination, self-loop first in group ----
_ADJ = {0: [1, 3, 5, 7], 1: [0, 2], 2: [1], 3: [0, 4], 4: [3],
        5: [0, 6], 6: [5], 7: [0, 8], 8: [7]}
EDGES = []          # (src, dst), sorted by dst, self-loop first
GRP_OFF = [0]       # group offsets into EDGES per dst
for _d in range(NN):
    EDGES.append((_d, _d))
    for _s in _ADJ[_d]:
        EDGES.append((_s, _d))
    GRP_OFF.append(len(EDGES))
NE = len(EDGES)     # 25
assert NE == 25


class Cfg:
    # per-edge-chunk adds not done on PE (TT add: DVE 1cpe, Pool ~2.9cpe)
    add_engines = ("vector", "gpsimd", "vector")
    # relu for SBUF-sourced adds (Pool tensor_scalar_max is legal)
    relu_engines = ("gpsimd", "gpsimd", "vector")
    # aggregation chain STT ops (DVE-only)
    stt_engines = ("vector",)
    # aggregation chain head (copy/mul with per-partition scale)
    first_engines = ("scalar", "vector")
    # psum->sbuf projection copies
    copy_engines = ("scalar", "scalar", "vector")
    # xl_gm psum->sbuf copies
    gm_copy_engines = ("scalar", "vector")
    proj_pair = 2                # 512-col groups per proj psum
    n_pe_adds = 20               # how many edge-chunk adds go on PE (from end)


def build_program(cfg: Cfg):
    nc = bacc.Bacc("TRN2", target_bir_lowering=False, debug=False)

    def eng(name):
        return {"vector": nc.vector, "gpsimd": nc.gpsimd,
                "scalar": nc.scalar}[name]

    def copy_op(ename, dst_ap, src_ap):
        if ename == "scalar":
            nc.scalar.copy(dst_ap, src_ap)
        else:
            eng(ename).tensor_copy(dst_ap, src_ap)

    # DRAM tensors
    xT_d = nc.dram_tensor("xT", [C, NBLK * NGT], BF16, kind="ExternalInput")
    wl_d = nc.dram_tensor("wl", [C, C], BF16, kind="ExternalInput")
    wr_d = nc.dram_tensor("wr", [C, C], BF16, kind="ExternalInput")
    attbl_d = nc.dram_tensor("attbl", [128, 2 * NE * NE], BF16,
                             kind="ExternalInput")
    wattbl_d = nc.dram_tensor("wattbl", [128, 2 * NN * 18], BF16,
                              kind="ExternalInput")
    vsel_d = nc.dram_tensor("vsel", [18, NE], F32, kind="ExternalInput")
    smat_d = nc.dram_tensor("smat", [NE, NE], F32, kind="ExternalInput")
    identf_d = nc.dram_tensor("identf", [128, 128], F32, kind="ExternalInput")
    identb_d = nc.dram_tensor("identb", [128, 128], BF16, kind="ExternalInput")
    out_d = nc.dram_tensor("out", [BC, NN * C], BF16, kind="ExternalOutput")

    with tile.TileContext(nc) as tc:
        with (
            tc.tile_pool(name="const", bufs=1) as cpool,
            tc.tile_pool(name="xin", bufs=2) as xpool,
            tc.tile_pool(name="proj", bufs=1) as prpool,
            tc.tile_pool(name="edge", bufs=6) as epool,
            tc.tile_pool(name="soft", bufs=2) as spool,
            tc.tile_pool(name="gm", bufs=8) as gmpool,
            tc.tile_pool(name="chain", bufs=4) as chpool,
            tc.tile_pool(name="outp", bufs=4) as opool,
            tc.tile_pool(name="ps_proj", bufs=2, space="PSUM") as ps_proj,
            tc.tile_pool(name="ps_gm", bufs=2, space="PSUM") as ps_gm,
            tc.tile_pool(name="ps_sc", bufs=1, space="PSUM") as ps_sc,
            tc.tile_pool(name="ps_misc", bufs=1, space="PSUM") as ps_misc,
        ):
            # ---- constants ----
            wl_sb = cpool.tile([128, 2 * C], BF16, tag="wl")
            wr_sb = cpool.tile([128, 2 * C], BF16, tag="wr")
            nc.sync.dma_start(wl_sb[:, 0:C], wl_d[0:128, :])
            nc.sync.dma_start(wl_sb[:, C:2 * C], wl_d[128:256, :])
            nc.sync.dma_start(wr_sb[:, 0:C], wr_d[0:128, :])
            nc.sync.dma_start(wr_sb[:, C:2 * C], wr_d[128:256, :])
            attbl_sb = cpool.tile([128, 2 * NE * NE], BF16, tag="attbl")
            nc.sync.dma_start(attbl_sb[:], attbl_d[:])
            wattbl_sb = cpool.tile([128, 2 * NN * 18], BF16, tag="wattbl")
            nc.sync.dma_start(wattbl_sb[:], wattbl_d[:])
            vsel_sb = cpool.tile([18, NE], F32, tag="vsel")
            nc.sync.dma_start(vsel_sb[:], vsel_d[:])
            smat_sb = cpool.tile([NE, NE], F32, tag="smat")
            nc.sync.dma_start(smat_sb[:], smat_d[:])
            identf_sb = cpool.tile([128, 128], F32, tag="identf")
            nc.sync.dma_start(identf_sb[:], identf_d[:])
            identb_sb = cpool.tile([128, 128], BF16, tag="identb")
            nc.sync.dma_start(identb_sb[:], identb_d[:])
            zero_sb = cpool.tile([128, G], BF16, tag="zero")
            nc.gpsimd.memset(zero_sb[:], 0.0)

            ci = 0    # proj copy engine cycler
            gci = 0   # gm copy engine cycler
            st_state = {}   # per-block saved tiles

            def make_proj_ops(b, xt):
                """20 closures: each runs the matmuls for one psum group and
                the psum->sbuf copy. Fills xlT/xrT maps in st_state[b]."""
                xlT = {}
                xrT = {}
                st_state[b]["xlT"] = xlT
                st_state[b]["xrT"] = xrT
                ops = []
                for (wsb, dest, nm) in ((wl_sb, xlT, "l"), (wr_sb, xrT, "r")):
                    for dch in range(2):
                        for s in range(NN):

                            def op(wsb=wsb, dest=dest, nm=nm, dch=dch, s=s):
                                nonlocal ci
                                ps = ps_proj.tile([128, G], F32,
                                                  tag="ps_proj")
                                nc.tensor.matmul(
                                    ps[:],
                                    wsb[:, dch * 128:dch * 128 + 128],
                                    xt[0][:, s * G:(s + 1) * G],
                                    start=True, stop=False)
                                nc.tensor.matmul(
                                    ps[:],
                                    wsb[:, C + dch * 128:C + dch * 128 + 128],
                                    xt[1][:, s * G:(s + 1) * G],
                                    start=False, stop=True)
                                dst = prpool.tile(
                                    [128, G], BF16, tag=f"p{nm}{dch}{s}")
                                dest[(dch, s)] = (dst, 0)
                                copy_op(
                                    cfg.copy_engines[ci % len(cfg.copy_engines)],
                                    dst[:], ps[:])
                                ci += 1

                            ops.append(op)
                return ops

            def make_tail_ops(bp):
                """Softmax tail + aggregation for block bp, as closures."""
                S = st_state[bp]
                ops = []

                def t_exp():
                    ex_sb = spool.tile([NE, G], F32, tag="ex")
                    S["ex_sb"] = ex_sb
                    nc.scalar.activation(ex_sb[:], S["sc_ps"][:],
                                         mybir.ActivationFunctionType.Exp)
                ops.append(t_exp)

                def t_den():
                    den_ps = ps_misc.tile([NE, G], F32, tag="ps_misc")
                    S["den_ps"] = den_ps
                    nc.tensor.matmul(den_ps[:], smat_sb[:], S["ex_sb"][:],
                                     start=True, stop=True)
                ops.append(t_den)

                def t_den_copy():
                    den_sb = spool.tile([NE, G], F32, tag="den")
                    S["den_sb"] = den_sb
                    nc.scalar.copy(den_sb[:], S["den_ps"][:])
                ops.append(t_den_copy)

                def t_transpose():
                    exT_ps = ps_misc.tile([128, 2 * NT * NE], F32,
                                          tag="ps_misc")
                    S["exT_ps"] = exT_ps
                    dT0 = NT * NE
                    for t in range(NT):
                        nc.tensor.transpose(
                            exT_ps[:, t * NE:(t + 1) * NE],
                            S["ex_sb"][:, t * 128:(t + 1) * 128],
                            identf_sb[0:NE, 0:NE])
                        nc.tensor.transpose(
                            exT_ps[:, dT0 + t * NE:dT0 + (t + 1) * NE],
                            S["den_sb"][:, t * 128:(t + 1) * 128],
                            identf_sb[0:NE, 0:NE])
                ops.append(t_transpose)

                def t_alpha():
                    dT0 = NT * NE
                    exT_ps = S["exT_ps"]
                    rdenT = spool.tile([128, NT * NE], F32, tag="rdenT")
                    nc.vector.reciprocal(rdenT[:], exT_ps[:, dT0:2 * dT0])
                    alphaT = spool.tile([128, NT * NE], F32, tag="alphaT")
                    nc.vector.tensor_tensor(
                        alphaT[:], exT_ps[:, 0:dT0], rdenT[:],
                        op=mybir.AluOpType.mult)
                    S["alphaT"] = alphaT
                ops.append(t_alpha)

                # aggregation chains, one closure per (t, d) group; the out
                # DMA for tile t goes right after its last group.
                gi = [0]

                def agg_group(t, d, out_t):
                    S_ = st_state[bp]
                    alphaT = S_["alphaT"]
                    xl_gm = S_["xl_gms"][t]
                    o0 = GRP_OFF[d]
                    k = GRP_OFF[d + 1] - o0
                    # chain head: m = xl_gm[src0] * alpha[e0]
                    prev = None
                    for j in range(k):
                        e = o0 + j
                        s = EDGES[e][0]
                        last = (j == k - 1)
                        if last:
                            dst_ap = out_t[:, d * C:(d + 1) * C]
                        else:
                            cht = chpool.tile([128, C], BF16,
                                              tag=f"ch{gi[0] % 4}",
                                              name="cht")
                            dst_ap = cht[:]
                        if j == 0:
                            ie = cfg.first_engines[gi[0] %
                                                   len(cfg.first_engines)]
                            if ie == "scalar":
                                nc.scalar.activation(
                                    dst_ap, xl_gm[:, s * C:(s + 1) * C],
                                    mybir.ActivationFunctionType.Copy,
                                    scale=alphaT[:, t * NE + e:
                                                 t * NE + e + 1])
                            else:
                                eng(ie).tensor_scalar_mul(
                                    dst_ap, xl_gm[:, s * C:(s + 1) * C],
                                    alphaT[:, t * NE + e:t * NE + e + 1])
                        else:
                            ge = cfg.stt_engines[gi[0] % len(cfg.stt_engines)]
                            eng(ge).scalar_tensor_tensor(
                                dst_ap,
                                xl_gm[:, s * C:(s + 1) * C],
                                alphaT[:, t * NE + e:t * NE + e + 1],
                                prev,
                                op0=mybir.AluOpType.mult,
                                op1=mybir.AluOpType.add)
                        gi[0] += 1
                        prev = dst_ap

                for t in range(NT):
                    def t_aggtile(t=t):
                        out_t = opool.tile([128, NN * C], BF16, tag="out_t")
                        for d in range(NN):
                            agg_group(t, d, out_t)
                        nc.sync.dma_start(
                            out_d[bp * G + t * 128:bp * G + (t + 1) * 128, :],
                            out_t[:])
                    ops.append(t_aggtile)
                return ops

            def emit_edge_phase(b):
                S = st_state[b]
                xt = S["xt"]
                xlT, xrT = S["xlT"], S["xrT"]

                # graph-major xl projection pieces, interleaved into the edge
                # loop so PE has ready work while score matmuls wait on
                # DVE-made inputs
                xl_gms = [gmpool.tile([128, NN * C], BF16, tag="xl_gm",
                                      name="xl_gm")
                          for _ in range(NT)]
                S["xl_gms"] = xl_gms

                def gm_piece(t, s, ce):
                    # xl_gm via PE transposes of the already-projected xlT
                    # (128-col passes, half the cycles of re-projecting)
                    nq = min(4, NN - s)
                    ps = ps_gm.tile([128, 1024], BF16, tag="ps_gmT")
                    for j in range(nq):
                        for dch in range(2):
                            lt, lo = xlT[(dch, s + j)]
                            nc.tensor.transpose(
                                ps[:, (j * 2 + dch) * 128:
                                   (j * 2 + dch + 1) * 128],
                                lt[:, lo + t * 128:lo + (t + 1) * 128],
                                identb_sb[:])
                    copy_op(ce, xl_gms[t][:, s * C:(s + nq) * C],
                            ps[:, 0:nq * C])

                gm_pieces = [(t, s) for t in range(NT)
                             for s in (0, 4, 8)]

                nonlocal gci
                # nu = [0.2*att@W_l | 0.2*att@W_r] . x  -> [18, G]
                nu_ps = ps_misc.tile([18, G], F32, tag="ps_misc")
                for s in range(NN):
                    for cc in range(2):
                        nc.tensor.matmul(
                            nu_ps[:],
                            wattbl_sb[:, (cc * NN + s) * 18:
                                      (cc * NN + s + 1) * 18],
                            xt[cc][:, s * G:(s + 1) * G],
                            start=(s == 0 and cc == 0),
                            stop=(s == NN - 1 and cc == 1))
                nu_sb = spool.tile([18, G], F32, tag="nu")
                nc.vector.tensor_copy(nu_sb[:], nu_ps[:])

                sc_ps = ps_sc.tile([NE, G], F32, tag="ps_sc")
                S["sc_ps"] = sc_ps
                ai = 0   # add engine cycler
                ri = 0   # relu engine cycler

                def relu_op(ename, dst_ap, src_ap):
                    if ename == "scalar":
                        nc.scalar.activation(
                            dst_ap, src_ap,
                            mybir.ActivationFunctionType.Relu)
                    elif ename == "vector":
                        nc.vector.tensor_scalar_max(dst_ap, src_ap, 0.0)
                    else:
                        nc.gpsimd.tensor_scalar_max(dst_ap, src_ap, 0.0)

                for e, (s, d) in enumerate(EDGES):
                    if e < len(gm_pieces):
                        gt, gs = gm_pieces[e]
                        gm_piece(gt, gs,
                                 cfg.gm_copy_engines[gci %
                                                     len(cfg.gm_copy_engines)])
                        gci += 1
                    for dch in range(2):
                        idx = e * 2 + dch
                        lt, lo = xlT[(dch, s)]
                        rt, ro = xrT[(dch, d)]
                        lk = epool.tile([128, G], BF16, tag="lk")
                        if idx >= 50 - cfg.n_pe_adds:
                            # PE add: s_e = I.T @ xl_s + I.T @ xr_d in psum;
                            # relu on ACT (reads PSUM)
                            ps = ps_gm.tile([128, 512], F32, tag="ps_pe")
                            nc.tensor.matmul(ps[:], identb_sb[:],
                                             lt[:, lo:lo + G],
                                             start=True, stop=False)
                            nc.tensor.matmul(ps[:], identb_sb[:],
                                             rt[:, ro:ro + G],
                                             start=False, stop=True)
                            nc.scalar.activation(
                                lk[:], ps[:],
                                mybir.ActivationFunctionType.Relu)
                        else:
                            st = epool.tile([128, G], BF16, tag="st")
                            ae = cfg.add_engines[ai % len(cfg.add_engines)]
                            ai += 1
                            eng(ae).tensor_tensor(
                                st[:], lt[:, lo:lo + G], rt[:, ro:ro + G],
                                op=mybir.AluOpType.add)
                            re = cfg.relu_engines[ri % len(cfg.relu_engines)]
                            ri += 1
                            relu_op(re, lk[:], st[:])
                        blk = (dch * NE + e) * NE
                        nc.tensor.matmul(
                            sc_ps[:], attbl_sb[:, blk:blk + NE], lk[:],
                            start=(e == 0 and dch == 0), stop=False)
                # linear 0.2*att*s part for all edges, joins last
                nc.tensor.matmul(sc_ps[:], vsel_sb[:], nu_sb[:],
                                 start=False, stop=True)

            # ---------------- main pipelined loop ----------------
            for b in range(NBLK + 1):
                proj_ops = []
                if b < NBLK:
                    st_state[b] = {}
                    xt = []
                    for cc in range(2):
                        t = xpool.tile([128, NGT], BF16, tag=f"xt{cc}")
                        nc.sync.dma_start(
                            t[:], xT_d[cc * 128:(cc + 1) * 128,
                                       b * NGT:(b + 1) * NGT])
                        xt.append(t)
                    st_state[b]["xt"] = xt
                    proj_ops = make_proj_ops(b, xt)
                tail_ops = make_tail_ops(b - 1) if b >= 1 else []

                # interleave: tail's cheap softmax steps early (between proj
                # pieces so PE never stalls on the exp->den->transpose chain),
                # agg tiles spread through the remaining proj pieces.
                sched = []
                pi, ti = 0, 0
                # softmax chain: exp, den, den_copy, transpose, alpha (5 ops)
                softmax_n = 5 if tail_ops else 0
                while pi < len(proj_ops) or ti < len(tail_ops):
                    if ti < softmax_n:
                        if pi < len(proj_ops):
                            sched.append(proj_ops[pi]); pi += 1
                        if pi < len(proj_ops):
                            sched.append(proj_ops[pi]); pi += 1
                        sched.append(tail_ops[ti]); ti += 1
                    elif ti < len(tail_ops):
                        if pi < len(proj_ops):
                            sched.append(proj_ops[pi]); pi += 1
                        if pi < len(proj_ops):
                            sched.append(proj_ops[pi]); pi += 1
                        sched.append(tail_ops[ti]); ti += 1
                    else:
                        sched.append(proj_ops[pi]); pi += 1
                for op in sched:
                    op()

                if b < NBLK:
                    emit_edge_phase(b)

    nc.compile()
    return nc


def make_host_inputs(x, W_l, W_r, att, cfg: Cfg):
    """Builds the per-core input maps (host-side sharding + layout prep)."""
    x = np.asarray(x, dtype=np.float32)
    W_l = np.ascontiguousarray(np.asarray(W_l, dtype=np.float32))
    W_r = np.ascontiguousarray(np.asarray(W_r, dtype=np.float32))
    att = np.asarray(att, dtype=np.float32)
    bf = ml_dtypes.bfloat16

    # 0.8-scaled att (x) onehot(e) stationary blocks for the relu part
    attbl = np.zeros((128, 2, NE, NE), dtype=np.float32)
    for dch in range(2):
        for e in range(NE):
            attbl[:, dch, e, e] = 0.8 * att[dch * 128:(dch + 1) * 128]
    attbl = attbl.reshape(128, 2 * NE * NE).astype(bf)

    # 0.2-scaled [att@W_l | att@W_r] one-hot-block stationaries for nu
    wattl = 0.2 * (W_l @ att)     # [256]
    wattr = 0.2 * (W_r @ att)
    wattbl = np.zeros((128, 2, NN, 18), dtype=np.float32)
    for cc in range(2):
        for s in range(NN):
            wattbl[:, cc, s, 2 * s] = wattl[cc * 128:(cc + 1) * 128]
            wattbl[:, cc, s, 2 * s + 1] = wattr[cc * 128:(cc + 1) * 128]
    wattbl = wattbl.reshape(128, 2 * NN * 18).astype(bf)

    # selector: score_e += nu_l[src_e] + nu_r[dst_e]
    vsel = np.zeros((18, NE), dtype=np.float32)
    for e, (s, d) in enumerate(EDGES):
        vsel[2 * s, e] = 1.0
        vsel[2 * d + 1, e] = 1.0

    smat = np.zeros((NE, NE), dtype=np.float32)
    for e1, (_s1, d1) in enumerate(EDGES):
        for e2, (_s2, d2) in enumerate(EDGES):
            if d1 == d2:
                smat[e1, e2] = 1.0

    ident = np.eye(128, dtype=np.float32)

    in_maps = []
    for c in range(N_CORES):
        xc = x[c * BC:(c + 1) * BC]                       # [BC, 9, 256]
        xT = np.ascontiguousarray(
            xc.reshape(NBLK, G, NN, C).transpose(3, 0, 2, 1).reshape(
                C, NBLK * NGT).astype(bf))
        in_maps.append({
            "xT": xT,
            "wl": W_l.astype(bf),
            "wr": W_r.astype(bf),
            "attbl": attbl,
            "wattbl": wattbl,
            "vsel": vsel,
            "smat": smat,
            "identf": ident,
            "identb": ident.astype(bf),
        })
    return in_maps


_CACHE = {}


def _cfg_key(cfg: Cfg):
    return (cfg.add_engines, cfg.relu_engines, cfg.stt_engines,
            cfg.first_engines, cfg.copy_engines, cfg.gm_copy_engines,
            cfg.proj_pair, cfg.n_pe_adds)


def _get_program(cfg: Cfg):
    key = _cfg_key(cfg)
    if key not in _CACHE:
        _CACHE[key] = build_program(cfg)
    return _CACHE[key]


def kernel(x, W_l, W_r, att, bias, cfg: Cfg = None, trace: bool = False,
           _results_holder: dict = None, **run_kwargs):
    cfg = cfg or Cfg()
    nc = _get_program(cfg)
    in_maps = make_host_inputs(x, W_l, W_r, att, cfg)
    res = run_bass_kernel_spmd(nc, in_maps, core_ids=list(range(N_CORES)),
                               trace=trace, **run_kwargs)
    if _results_holder is not None:
        _results_holder["res"] = res
    outs = [np.asarray(r["out"], dtype=np.float32).reshape(BC, NN, C)
            for r in res.results]
    out = np.concatenate(outs, axis=0)
    bias = np.asarray(bias, dtype=np.float32)
    if np.any(bias):
        out = out + bias
    return out.astype(np.float32)


# revision 20
# speedup vs baseline: 2.4242x; 2.4242x over previous
"""GATv2Conv batched-graph kernel for Trainium2 — channel-major aggregation.

Differences from the graph-major variant: after softmax, alpha rows
[25, G] are broadcast across partitions with tiny PE matmuls
(ones[1,128]^T @ alpha_row) into PSUM, and the aggregation runs in
channel-major layout as plain tensor_tensor ops (1 cycle/elem on DVE):
msg_e = xlT[src] * alphaB_e (in1 read straight from PSUM, no copy),
group-summed with non-RMW TT-add chains into channel-major output
tiles. This kills the graph-major xl_gm tensor, its PSUM->SBUF copies,
the alpha transposes, and the 2-cycle/elem STT chains. Output leaves
the device channel-major; the host transposes it back.
"""

import sys

if "/opt/trn_rl_repo" not in sys.path:
    sys.path.insert(0, "/opt/trn_rl_repo")

import numpy as np
import ml_dtypes

import concourse.bass as bass
import concourse.bacc as bacc
import concourse.mybir as mybir
from concourse import tile
from concourse.bass_utils import run_bass_kernel_spmd

F32 = mybir.dt.float32
BF16 = mybir.dt.bfloat16

N_CORES = 8
B_TOTAL = 16384
BC = B_TOTAL // N_CORES          # graphs per core
NN = 9                           # nodes per graph
C = 256                          # channels
G = 512                          # graphs per block
NBLK = BC // G                   # blocks per core
NT = G // 128
NGT = NN * G                     # columns per (chunk, block)

_ADJ = {0: [1, 3, 5, 7], 1: [0, 2], 2: [1], 3: [0, 4], 4: [3],
        5: [0, 6], 6: [5], 7: [0, 8], 8: [7]}
EDGES = []          # (src, dst), sorted by dst, self-loop first
GRP_OFF = [0]
for _d in range(NN):
    EDGES.append((_d, _d))
    for _s in _ADJ[_d]:
        EDGES.append((_s, _d))
    GRP_OFF.append(len(EDGES))
NE = len(EDGES)     # 25
assert NE == 25


class Cfg:
    # NB: Pool (gpsimd) is only usable for tensor_tensor add/mult on SBUF
    # (~1.4us per [128,512] op); its tensor_scalar is a ~9us software loop.
    add_engines = ("vector", "vector", "gpsimd")
    relu_engines = ("scalar", "scalar", "vector")
    copy_engines = ("scalar", "vector", "scalar")
    mult_engines = ("vector",)
    aggadd_engines = ("gpsimd", "vector")
    n_pe_adds = 20


def build_program(cfg: Cfg):
    nc = bacc.Bacc("TRN2", target_bir_lowering=False, debug=False)

    def eng(name):
        return {"vector": nc.vector, "gpsimd": nc.gpsimd,
                "scalar": nc.scalar}[name]

    def copy_op(ename, dst_ap, src_ap):
        if ename == "scalar":
            nc.scalar.copy(dst_ap, src_ap)
        else:
            eng(ename).tensor_copy(dst_ap, src_ap)

    xT_d = nc.dram_tensor("xT", [C, NBLK * NGT], BF16, kind="ExternalInput")
    wl_d = nc.dram_tensor("wl", [C, C], BF16, kind="ExternalInput")
    wr_d = nc.dram_tensor("wr", [C, C], BF16, kind="ExternalInput")
    attbl_d = nc.dram_tensor("attbl", [128, 2 * NE * NE], BF16,
                             kind="ExternalInput")
    wattbl_d = nc.dram_tensor("wattbl", [128, 2 * NN * 18], BF16,
                              kind="ExternalInput")
    vsel_d = nc.dram_tensor("vsel", [18, NE], F32, kind="ExternalInput")
    smat_d = nc.dram_tensor("smat", [NE, NE], F32, kind="ExternalInput")
    identb_d = nc.dram_tensor("identb", [128, 128], BF16, kind="ExternalInput")
    out_d = nc.dram_tensor("out", [C, NBLK * NGT], BF16,
                           kind="ExternalOutput")

    with tile.TileContext(nc) as tc:
        with (
            tc.tile_pool(name="const", bufs=1) as cpool,
            tc.tile_pool(name="xin", bufs=2) as xpool,
            tc.tile_pool(name="proj", bufs=1) as prpool,
            tc.tile_pool(name="edge", bufs=6) as epool,
            tc.tile_pool(name="soft", bufs=2) as spool,
            tc.tile_pool(name="msg", bufs=4) as mpool,
            tc.tile_pool(name="outp", bufs=1) as opool,
            tc.tile_pool(name="alfp", bufs=1) as afpool,
            tc.tile_pool(name="ps_proj", bufs=2, space="PSUM") as ps_proj,
            tc.tile_pool(name="ps_pe", bufs=2, space="PSUM") as ps_pe,
            tc.tile_pool(name="ps_bc", bufs=2, space="PSUM") as ps_bc,
            tc.tile_pool(name="ps_sc", bufs=1, space="PSUM") as ps_sc,
            tc.tile_pool(name="ps_misc", bufs=1, space="PSUM") as ps_misc,
        ):
            wl_sb = cpool.tile([128, 2 * C], BF16, tag="wl")
            wr_sb = cpool.tile([128, 2 * C], BF16, tag="wr")
            nc.sync.dma_start(wl_sb[:, 0:C], wl_d[0:128, :])
            nc.sync.dma_start(wl_sb[:, C:2 * C], wl_d[128:256, :])
            nc.sync.dma_start(wr_sb[:, 0:C], wr_d[0:128, :])
            nc.sync.dma_start(wr_sb[:, C:2 * C], wr_d[128:256, :])
            attbl_sb = cpool.tile([128, 2 * NE * NE], BF16, tag="attbl")
            nc.sync.dma_start(attbl_sb[:], attbl_d[:])
            wattbl_sb = cpool.tile([128, 2 * NN * 18], BF16, tag="wattbl")
            nc.sync.dma_start(wattbl_sb[:], wattbl_d[:])
            vsel_sb = cpool.tile([18, NE], F32, tag="vsel")
            nc.sync.dma_start(vsel_sb[:], vsel_d[:])
            smat_sb = cpool.tile([NE, NE], F32, tag="smat")
            nc.sync.dma_start(smat_sb[:], smat_d[:])
            identb_sb = cpool.tile([128, 128], BF16, tag="identb")
            nc.sync.dma_start(identb_sb[:], identb_d[:])
            ones_sb = cpool.tile([65, 128], F32, tag="ones")
            nc.gpsimd.memset(ones_sb[:], 1.0)

            ci = 0
            st_state = {}

            def make_proj_ops(b, xt):
                xlT = {}
                xrT = {}
                st_state[b]["xlT"] = xlT
                st_state[b]["xrT"] = xrT
                ops = []
                for (wsb, dest, nm, nbufs) in (
                        (wl_sb, xlT, "l", 2), (wr_sb, xrT, "r", 1)):
                    for dch in range(2):
                        for s in range(NN):

                            def op(wsb=wsb, dest=dest, nm=nm, dch=dch, s=s,
                                   nbufs=nbufs):
                                nonlocal ci
                                ps = ps_proj.tile([128, G], F32,
                                                  tag="ps_proj")
                                nc.tensor.matmul(
                                    ps[:],
                                    wsb[:, dch * 128:dch * 128 + 128],
                                    xt[0][:, s * G:(s + 1) * G],
                                    start=True, stop=False)
                                nc.tensor.matmul(
                                    ps[:],
                                    wsb[:, C + dch * 128:C + dch * 128 + 128],
                                    xt[1][:, s * G:(s + 1) * G],
                                    start=False, stop=True)
                                dst = prpool.tile(
                                    [128, G], BF16, tag=f"p{nm}{dch}{s}",
                                    bufs=nbufs)
                                dest[(dch, s)] = dst
                                copy_op(
                                    cfg.copy_engines[ci % len(cfg.copy_engines)],
                                    dst[:], ps[:])
                                ci += 1

                            ops.append(op)
                return ops

            def make_tail_ops(bp):
                """Softmax + channel-major aggregation for block bp."""
                S = st_state[bp]
                ops = []

                def t_exp():
                    ex_sb = spool.tile([NE, G], F32, tag="ex")
                    S["ex_sb"] = ex_sb
                    nc.scalar.activation(ex_sb[:], S["sc_ps"][:],
                                         mybir.ActivationFunctionType.Exp)
                ops.append(t_exp)

                def t_den():
                    den_ps = ps_misc.tile([NE, G], F32, tag="ps_misc")
                    S["den_ps"] = den_ps
                    nc.tensor.matmul(den_ps[:], smat_sb[:], S["ex_sb"][:],
                                     start=True, stop=True)
                ops.append(t_den)

                def t_den_copy():
                    den_sb = spool.tile([NE, G], F32, tag="den")
                    S["den_sb"] = den_sb
                    nc.scalar.copy(den_sb[:], S["den_ps"][:])
                ops.append(t_den_copy)

                def t_alpha():
                    rden = spool.tile([NE, G], F32, tag="rden")
                    nc.vector.reciprocal_approx_fast(rden[:], S["den_sb"][:])
                    al = spool.tile([NE, G], F32, tag="al")
                    nc.vector.tensor_tensor(al[:], S["ex_sb"][:], rden[:],
                                            op=mybir.AluOpType.mult)
                    S["al"] = al
                ops.append(t_alpha)

                def t_alflat():
                    # PE moving operands must start at partition 0/32/64:
                    # reshuffle alpha rows onto those partitions via DMA
                    al = S["al"]
                    alf = afpool.tile([65, 9 * G], F32, tag="alf")
                    nc.sync.dma_start(alf[0:1, 0:9 * G], al[0:9, :])
                    nc.sync.dma_start(alf[32:33, 0:8 * G], al[9:17, :])
                    nc.sync.dma_start(alf[64:65, 0:8 * G], al[17:25, :])
                    S["alf"] = alf
                ops.append(t_alflat)

                # channel-major output tiles
                def t_mkout():
                    outs = []
                    for cc in range(2):
                        ot = opool.tile([128, NN * G], BF16, tag=f"out{cc}",
                                        name="ot")
                        outs.append(ot)
                    S["outs"] = outs
                ops.append(t_mkout)

                ai = [0]

                def agg_edge(e):
                    S_ = st_state[bp]
                    s, d = EDGES[e]
                    first = (e == GRP_OFF[d])
                    last = (e == GRP_OFF[d + 1] - 1)
                    bc = ps_bc.tile([128, G], F32, tag="ps_bc")
                    if e < 9:
                        p0, off = 0, e * G
                    elif e < 17:
                        p0, off = 32, (e - 9) * G
                    else:
                        p0, off = 64, (e - 17) * G
                    nc.tensor.matmul(bc[:], ones_sb[p0:p0 + 1, :],
                                     S_["alf"][p0:p0 + 1, off:off + G],
                                     start=True, stop=True)
                    for cc in range(2):
                        m = mpool.tile([128, G], BF16,
                                       tag=f"m{ai[0] % 5}", name="m")
                        me = cfg.mult_engines[ai[0] % len(cfg.mult_engines)]
                        eng(me).tensor_tensor(
                            m[:], S_["xlT"][(cc, s)][:], bc[:],
                            op=mybir.AluOpType.mult)
                        if first:
                            S_[("acc", cc, d)] = m
                        else:
                            prev = S_[("acc", cc, d)]
                            if last:
                                dst_ap = S_["outs"][cc][:,
                                                        d * G:(d + 1) * G]
                            else:
                                nt = mpool.tile([128, G], BF16,
                                                tag=f"m{(ai[0] + 2) % 5}",
                                                name="nt")
                                dst_ap = nt[:]
                                S_[("acc", cc, d)] = nt
                            ae = cfg.aggadd_engines[ai[0] %
                                                    len(cfg.aggadd_engines)]
                            eng(ae).tensor_tensor(
                                dst_ap, prev[:], m[:],
                                op=mybir.AluOpType.add)
                        ai[0] += 1

                for e in range(NE):
                    ops.append(lambda e=e: agg_edge(e))

                def t_outdma():
                    for cc in range(2):
                        nc.sync.dma_start(
                            out_d[cc * 128:(cc + 1) * 128,
                                  bp * NGT:(bp + 1) * NGT],
                            S["outs"][cc][:])
                ops.append(t_outdma)
                return ops

            def emit_edge_phase(b):
                S = st_state[b]
                xt = S["xt"]
                xlT, xrT = S["xlT"], S["xrT"]

                # nu = [0.2*att@W_l | 0.2*att@W_r] . x  -> [18, G]
                nu_ps = ps_misc.tile([18, G], F32, tag="ps_misc")
                for s in range(NN):
                    for cc in range(2):
                        nc.tensor.matmul(
                            nu_ps[:],
                            wattbl_sb[:, (cc * NN + s) * 18:
                                      (cc * NN + s + 1) * 18],
                            xt[cc][:, s * G:(s + 1) * G],
                            start=(s == 0 and cc == 0),
                            stop=(s == NN - 1 and cc == 1))
                nu_sb = spool.tile([18, G], F32, tag="nu")
                nc.vector.tensor_copy(nu_sb[:], nu_ps[:])

                sc_ps = ps_sc.tile([NE, G], F32, tag="ps_sc")
                S["sc_ps"] = sc_ps
                ai = 0
                ri = 0

                def relu_op(ename, dst_ap, src_ap):
                    if ename == "scalar":
                        nc.scalar.activation(
                            dst_ap, src_ap,
                            mybir.ActivationFunctionType.Relu)
                    elif ename == "vector":
                        nc.vector.tensor_scalar_max(dst_ap, src_ap, 0.0)
                    else:
                        nc.gpsimd.tensor_scalar_max(dst_ap, src_ap, 0.0)

                for e, (s, d) in enumerate(EDGES):
                    for dch in range(2):
                        idx = e * 2 + dch
                        lt = xlT[(dch, s)]
                        rt = xrT[(dch, d)]
                        lk = epool.tile([128, G], BF16, tag="lk")
                        if idx >= 50 - cfg.n_pe_adds:
                            ps = ps_pe.tile([128, G], F32, tag="ps_pe")
                            nc.tensor.matmul(ps[:], identb_sb[:], lt[:],
                                             start=True, stop=False)
                            nc.tensor.matmul(ps[:], identb_sb[:], rt[:],
                                             start=False, stop=True)
                            nc.scalar.activation(
                                lk[:], ps[:],
                                mybir.ActivationFunctionType.Relu)
                        else:
                            st = epool.tile([128, G], BF16, tag="st")
                            ae = cfg.add_engines[ai % len(cfg.add_engines)]
                            ai += 1
                            eng(ae).tensor_tensor(
                                st[:], lt[:], rt[:],
                                op=mybir.AluOpType.add)
                            re = cfg.relu_engines[ri % len(cfg.relu_engines)]
                            ri += 1
                            relu_op(re, lk[:], st[:])
                        blk = (dch * NE + e) * NE
                        nc.tensor.matmul(
                            sc_ps[:], attbl_sb[:, blk:blk + NE], lk[:],
                            start=(e == 0 and dch == 0), stop=False)
                nc.tensor.matmul(sc_ps[:], vsel_sb[:], nu_sb[:],
                                 start=False, stop=True)

            # ---------------- main pipelined loop ----------------
            for b in range(NBLK + 1):
                proj_ops = []
                if b < NBLK:
                    st_state[b] = {}
                    xt = []
                    for cc in range(2):
                        t = xpool.tile([128, NGT], BF16, tag=f"xt{cc}")
                        nc.sync.dma_start(
                            t[:], xT_d[cc * 128:(cc + 1) * 128,
                                       b * NGT:(b + 1) * NGT])
                        xt.append(t)
                    st_state[b]["xt"] = xt
                    proj_ops = make_proj_ops(b, xt)
                tail_ops = make_tail_ops(b - 1) if b >= 1 else []

                sched = []
                pi, ti = 0, 0
                while pi < len(proj_ops) or ti < len(tail_ops):
                    if ti < len(tail_ops):
                        if pi < len(proj_ops):
                            sched.append(proj_ops[pi]); pi += 1
                        sched.append(tail_ops[ti]); ti += 1
                    else:
                        sched.append(proj_ops[pi]); pi += 1
                for op in sched:
                    op()

                if b < NBLK:
                    emit_edge_phase(b)

    nc.compile()
    return nc


def make_host_inputs(x, W_l, W_r, att, cfg: Cfg):
    x = np.asarray(x, dtype=np.float32)
    W_l = np.ascontiguousarray(np.asarray(W_l, dtype=np.float32))
    W_r = np.ascontiguousarray(np.asarray(W_r, dtype=np.float32))
    att = np.asarray(att, dtype=np.float32)
    bf = ml_dtypes.bfloat16

    attbl = np.zeros((128, 2, NE, NE), dtype=np.float32)
    for dch in range(2):
        for e in range(NE):
            attbl[:, dch, e, e] = 0.8 * att[dch * 128:(dch + 1) * 128]
    attbl = attbl.reshape(128, 2 * NE * NE).astype(bf)

    wattl = 0.2 * (W_l @ att)
    wattr = 0.2 * (W_r @ att)
    wattbl = np.zeros((128, 2, NN, 18), dtype=np.float32)
    for cc in range(2):
        for s in range(NN):
            wattbl[:, cc, s, 2 * s] = wattl[cc * 128:(cc + 1) * 128]
            wattbl[:, cc, s, 2 * s + 1] = wattr[cc * 128:(cc + 1) * 128]
    wattbl = wattbl.reshape(128, 2 * NN * 18).astype(bf)

    vsel = np.zeros((18, NE), dtype=np.float32)
    for e, (s, d) in enumerate(EDGES):
        vsel[2 * s, e] = 1.0
        vsel[2 * d + 1, e] = 1.0

    smat = np.zeros((NE, NE), dtype=np.float32)
    for e1, (_s1, d1) in enumerate(EDGES):
        for e2, (_s2, d2) in enumerate(EDGES):
            if d1 == d2:
                smat[e1, e2] = 1.0

    ident = np.eye(128, dtype=np.float32)

    in_maps = []
    for c in range(N_CORES):
        xc = x[c * BC:(c + 1) * BC]
        xT = np.ascontiguousarray(
            xc.reshape(NBLK, G, NN, C).transpose(3, 0, 2, 1).reshape(
                C, NBLK * NGT).astype(bf))
        in_maps.append({
            "xT": xT,
            "wl": W_l.astype(bf),
            "wr": W_r.astype(bf),
            "attbl": attbl,
            "wattbl": wattbl,
            "vsel": vsel,
            "smat": smat,
            "identb": ident.astype(bf),
        })
    return in_maps


_CACHE = {}


def _cfg_key(cfg: Cfg):
    return (cfg.add_engines, cfg.relu_engines, cfg.copy_engines,
            cfg.mult_engines, cfg.aggadd_engines, cfg.n_pe_adds)


def _get_program(cfg: Cfg):
    key = _cfg_key(cfg)
    if key not in _CACHE:
        _CACHE[key] = build_program(cfg)
    return _CACHE[key]


def kernel(x, W_l, W_r, att, bias, cfg: Cfg = None, trace: bool = False,
           _results_holder: dict = None, **run_kwargs):
    cfg = cfg or Cfg()
    nc = _get_program(cfg)
    in_maps = make_host_inputs(x, W_l, W_r, att, cfg)
    res = run_bass_kernel_spmd(nc, in_maps, core_ids=list(range(N_CORES)),
                               trace=trace, **run_kwargs)
    if _results_holder is not None:
        _results_holder["res"] = res
    outs = []
    for r in res.results:
        o = np.asarray(r["out"], dtype=np.float32)      # [C, NBLK*NGT]
        o = o.reshape(C, NBLK, NN, G).transpose(1, 3, 2, 0).reshape(
            BC, NN, C)
        outs.append(o)
    out = np.concatenate(outs, axis=0)
    bias = np.asarray(bias, dtype=np.float32)
    if np.any(bias):
        out = out + bias
    return out.astype(np.float32)


# revision 21
# speedup vs baseline: 2.6863x; 1.1081x over previous
"""GATv2Conv batched-graph kernel for Trainium2 (8 NeuronCores, data-parallel).

Baseline + xl_gm built by PE transposes of xlT (saves 9.2k PE cycles/block).
"""

import sys

if "/opt/trn_rl_repo" not in sys.path:
    sys.path.insert(0, "/opt/trn_rl_repo")

import numpy as np
import ml_dtypes

import concourse.bass as bass
import concourse.bacc as bacc
import concourse.mybir as mybir
from concourse import tile
from concourse.bass_utils import run_bass_kernel_spmd

F32 = mybir.dt.float32
BF16 = mybir.dt.bfloat16

N_CORES = 8
B_TOTAL = 16384
BC = B_TOTAL // N_CORES          # graphs per core
NN = 9                           # nodes per graph
C = 256                          # channels
G = 512                          # graphs per block
NBLK = BC // G                   # blocks per core
NT = G // 128                    # 128-graph subtiles per block
NGT = NN * G                     # columns per (chunk, block)

# ---- static edge list, grouped by destination, self-loop first in group ----
_ADJ = {0: [1, 3, 5, 7], 1: [0, 2], 2: [1], 3: [0, 4], 4: [3],
        5: [0, 6], 6: [5], 7: [0, 8], 8: [7]}
EDGES = []          # (src, dst), sorted by dst, self-loop first
GRP_OFF = [0]       # group offsets into EDGES per dst
for _d in range(NN):
    EDGES.append((_d, _d))
    for _s in _ADJ[_d]:
        EDGES.append((_s, _d))
    GRP_OFF.append(len(EDGES))
NE = len(EDGES)     # 25
assert NE == 25


class Cfg:
    add_engines = ("vector", "gpsimd")       # per-edge adds round robin
    relu_engines = ("vector", "scalar")      # per-edge relu round robin
    agg_engines = ("vector",)                # agg STT round robin (AP scalar
                                             # ops are DVE-only)
    copy_engines = ("scalar",)               # psum->sbuf copies (ACT gap-fills)
    aggi_engines = ("scalar", "scalar", "vector")  # agg init: copy with scale
    proj_pair = 2                            # 512-col groups per proj psum


def build_program(cfg: Cfg):
    nc = bacc.Bacc("TRN2", target_bir_lowering=False, debug=False)

    def eng(name):
        return {"vector": nc.vector, "gpsimd": nc.gpsimd,
                "scalar": nc.scalar}[name]

    def copy_op(ename, dst_ap, src_ap):
        if ename == "scalar":
            nc.scalar.copy(dst_ap, src_ap)
        else:
            eng(ename).tensor_copy(dst_ap, src_ap)

    # DRAM tensors
    xT_d = nc.dram_tensor("xT", [C, NBLK * NGT], BF16, kind="ExternalInput")
    wl_d = nc.dram_tensor("wl", [C, C], BF16, kind="ExternalInput")
    wr_d = nc.dram_tensor("wr", [C, C], BF16, kind="ExternalInput")
    attbl_d = nc.dram_tensor("attbl", [128, 2 * NE * NE], BF16,
                             kind="ExternalInput")
    wattbl_d = nc.dram_tensor("wattbl", [128, 2 * NN * 18], BF16,
                              kind="ExternalInput")
    vsel_d = nc.dram_tensor("vsel", [18, NE], F32, kind="ExternalInput")
    smat_d = nc.dram_tensor("smat", [NE, NE], F32, kind="ExternalInput")
    identf_d = nc.dram_tensor("identf", [128, 128], F32, kind="ExternalInput")
    identb_d = nc.dram_tensor("identb", [128, 128], BF16, kind="ExternalInput")
    out_d = nc.dram_tensor("out", [BC, NN * C], BF16, kind="ExternalOutput")

    with tile.TileContext(nc) as tc:
        with (
            tc.tile_pool(name="const", bufs=1) as cpool,
            tc.tile_pool(name="xin", bufs=2) as xpool,
            tc.tile_pool(name="proj", bufs=2) as prpool,
            tc.tile_pool(name="edge", bufs=10) as epool,
            tc.tile_pool(name="soft", bufs=3) as spool,
            tc.tile_pool(name="gm", bufs=6) as gmpool,
            tc.tile_pool(name="outp", bufs=4) as opool,
            tc.tile_pool(name="ps_proj", bufs=2, space="PSUM") as ps_proj,
            tc.tile_pool(name="ps_gm", bufs=2, space="PSUM") as ps_gm,
            tc.tile_pool(name="ps_sc", bufs=1, space="PSUM") as ps_sc,
            tc.tile_pool(name="ps_misc", bufs=1, space="PSUM") as ps_misc,
        ):
            # ---- constants ----
            wl_sb = cpool.tile([128, 2 * C], BF16, tag="wl")
            wr_sb = cpool.tile([128, 2 * C], BF16, tag="wr")
            nc.sync.dma_start(wl_sb[:, 0:C], wl_d[0:128, :])
            nc.sync.dma_start(wl_sb[:, C:2 * C], wl_d[128:256, :])
            nc.sync.dma_start(wr_sb[:, 0:C], wr_d[0:128, :])
            nc.sync.dma_start(wr_sb[:, C:2 * C], wr_d[128:256, :])
            attbl_sb = cpool.tile([128, 2 * NE * NE], BF16, tag="attbl")
            nc.sync.dma_start(attbl_sb[:], attbl_d[:])
            wattbl_sb = cpool.tile([128, 2 * NN * 18], BF16, tag="wattbl")
            nc.sync.dma_start(wattbl_sb[:], wattbl_d[:])
            vsel_sb = cpool.tile([18, NE], F32, tag="vsel")
            nc.sync.dma_start(vsel_sb[:], vsel_d[:])
            smat_sb = cpool.tile([NE, NE], F32, tag="smat")
            nc.sync.dma_start(smat_sb[:], smat_d[:])
            identf_sb = cpool.tile([128, 128], F32, tag="identf")
            nc.sync.dma_start(identf_sb[:], identf_d[:])
            identb_sb = cpool.tile([128, 128], BF16, tag="identb")
            nc.sync.dma_start(identb_sb[:], identb_d[:])
            zero_sb = cpool.tile([128, G], BF16, tag="zero")
            nc.gpsimd.memset(zero_sb[:], 0.0)

            ci = 0   # copy engine cycler
            for b in range(NBLK):
                # ---- load xT block: 2 channel-chunks [128, NN*G] ----
                xt = []
                for cc in range(2):
                    t = xpool.tile([128, NGT], BF16, tag=f"xt{cc}")
                    nc.sync.dma_start(
                        t[:], xT_d[cc * 128:(cc + 1) * 128,
                                   b * NGT:(b + 1) * NGT])
                    xt.append(t)

                # ---- channel-major projections, one tile per psum copy ----
                # fine-grained tiles so edge adds gate on single copies
                xlT = {}   # (dch, s) -> (tile, col offset)
                xrT = {}
                for (wsb, dest, nm) in ((wl_sb, xlT, "l"), (wr_sb, xrT, "r")):
                    for dch in range(2):
                        s = 0
                        while s < NN:
                            npair = min(cfg.proj_pair, NN - s)
                            ps = ps_proj.tile([128, cfg.proj_pair * G], F32,
                                              tag="ps_proj")
                            for j in range(npair):
                                nc.tensor.matmul(
                                    ps[:, j * G:(j + 1) * G],
                                    wsb[:, dch * 128:dch * 128 + 128],
                                    xt[0][:, (s + j) * G:(s + j + 1) * G],
                                    start=True, stop=False)
                                nc.tensor.matmul(
                                    ps[:, j * G:(j + 1) * G],
                                    wsb[:, C + dch * 128:C + dch * 128 + 128],
                                    xt[1][:, (s + j) * G:(s + j + 1) * G],
                                    start=False, stop=True)
                            dst = prpool.tile(
                                [128, npair * G], BF16,
                                tag=f"p{nm}{dch}{s}")
                            for j in range(npair):
                                dest[(dch, s + j)] = (dst, j * G)
                            copy_op(cfg.copy_engines[ci % len(cfg.copy_engines)],
                                    dst[:], ps[:, 0:npair * G])
                            ci += 1
                            s += npair

                # ---- graph-major xl for aggregation (stationary = xT) ----
                # emitted as closures interleaved into the edge loop so PE
                # has ready work while score matmuls wait on DVE-made inputs
                xl_gms = [gmpool.tile([128, NN * C], BF16, tag="xl_gm",
                                      name="xl_gm")
                          for _ in range(NT)]

                def gm_piece(t, s, ce):
                    # xl_gm via PE transposes of the already-projected xlT
                    # (128-col passes, half the cycles of re-projecting)
                    nq = min(4, NN - s)
                    ps = ps_gm.tile([128, 1024], BF16, tag="ps_gmT")
                    for j in range(nq):
                        for dch in range(2):
                            lt, lo = xlT[(dch, s + j)]
                            nc.tensor.transpose(
                                ps[:, (j * 2 + dch) * 128:
                                   (j * 2 + dch + 1) * 128],
                                lt[:, lo + t * 128:lo + (t + 1) * 128],
                                identb_sb[:])
                    copy_op(ce, xl_gms[t][:, s * C:(s + nq) * C],
                            ps[:, 0:nq * C])

                gm_pieces = [(t, s) for t in range(NT)
                             for s in (0, 4, 8)]

                # ---- nu = [0.2*att@W_l | 0.2*att@W_r] . x per node ----
                nu_ps = ps_misc.tile([18, G], F32, tag="ps_misc")
                for s in range(NN):
                    for cc in range(2):
                        nc.tensor.matmul(
                            nu_ps[:],
                            wattbl_sb[:, (cc * NN + s) * 18:
                                      (cc * NN + s + 1) * 18],
                            xt[cc][:, s * G:(s + 1) * G],
                            start=(s == 0 and cc == 0),
                            stop=(s == NN - 1 and cc == 1))
                nu_sb = spool.tile([18, G], F32, tag="nu")
                nc.vector.tensor_copy(nu_sb[:], nu_ps[:])

                # ---- edge phase: adds + relu + score matmuls ----
                sc_ps = ps_sc.tile([NE, G], F32, tag="ps_sc")
                for e, (s, d) in enumerate(EDGES):
                    if e < len(gm_pieces):
                        gt, gs = gm_pieces[e]
                        gm_piece(gt, gs,
                                 cfg.copy_engines[ci % len(cfg.copy_engines)])
                        ci += 1
                    for dch in range(2):
                        idx = e * 2 + dch
                        st = epool.tile([128, G], BF16, tag="st")
                        ae = cfg.add_engines[idx % len(cfg.add_engines)]
                        lt, lo = xlT[(dch, s)]
                        rt, ro = xrT[(dch, d)]
                        eng(ae).tensor_tensor(
                            st[:], lt[:, lo:lo + G], rt[:, ro:ro + G],
                            op=mybir.AluOpType.add)
                        lk = epool.tile([128, G], BF16, tag="lk")
                        re = cfg.relu_engines[idx % len(cfg.relu_engines)]
                        if re == "scalar":
                            nc.scalar.activation(
                                lk[:], st[:],
                                mybir.ActivationFunctionType.Relu)
                        else:
                            eng(re).tensor_scalar_max(lk[:], st[:], 0.0)
                        blk = (dch * NE + e) * NE
                        nc.tensor.matmul(
                            sc_ps[:], attbl_sb[:, blk:blk + NE], lk[:],
                            start=(e == 0 and dch == 0), stop=False)
                # linear 0.2*att*s part for all edges, joins last
                nc.tensor.matmul(sc_ps[:], vsel_sb[:], nu_sb[:],
                                 start=False, stop=True)

                # ---- softmax (no max-subtraction; scores are O(1)) ----
                ex_sb = spool.tile([NE, G], F32, tag="ex")
                nc.scalar.activation(ex_sb[:], sc_ps[:],
                                     mybir.ActivationFunctionType.Exp)
                # per-edge gathered denominator: M[e',e] = [dst match]
                den_ps = ps_misc.tile([NE, G], F32, tag="ps_misc")
                nc.tensor.matmul(den_ps[:], smat_sb[:], ex_sb[:],
                                 start=True, stop=True)
                den_sb = spool.tile([NE, G], F32, tag="den")
                nc.scalar.copy(den_sb[:], den_ps[:])

                # transposed per-128-graph alpha: one TT per block
                exT_ps = ps_misc.tile([128, 2 * NT * NE], F32,
                                      tag="ps_misc")
                dT0 = NT * NE
                for t in range(NT):
                    nc.tensor.transpose(
                        exT_ps[:, t * NE:(t + 1) * NE],
                        ex_sb[:, t * 128:(t + 1) * 128],
                        identf_sb[0:NE, 0:NE])
                    nc.tensor.transpose(
                        exT_ps[:, dT0 + t * NE:dT0 + (t + 1) * NE],
                        den_sb[:, t * 128:(t + 1) * 128],
                        identf_sb[0:NE, 0:NE])
                rdenT = spool.tile([128, NT * NE], F32, tag="rdenT")
                nc.vector.reciprocal(rdenT[:], exT_ps[:, dT0:dT0 + 2 * dT0 - dT0])
                alphaT = spool.tile([128, NT * NE], F32, tag="alphaT")
                nc.vector.tensor_tensor(
                    alphaT[:], exT_ps[:, 0:dT0], rdenT[:],
                    op=mybir.AluOpType.mult)

                # ---- aggregation in graph-major ----
                gi = 0
                for t in range(NT):
                    xl_gm = xl_gms[t]
                    out_t = opool.tile([128, NN * C], BF16, tag="out_t")
                    for d in range(NN):
                        o0 = GRP_OFF[d]
                        ie = cfg.aggi_engines[gi % len(cfg.aggi_engines)]
                        if ie == "scalar":
                            nc.scalar.activation(
                                out_t[:, d * C:(d + 1) * C],
                                xl_gm[:, d * C:(d + 1) * C],
                                mybir.ActivationFunctionType.Copy,
                                scale=alphaT[:, t * NE + o0:t * NE + o0 + 1])
                        else:
                            eng(ie).tensor_scalar_mul(
                                out_t[:, d * C:(d + 1) * C],
                                xl_gm[:, d * C:(d + 1) * C],
                                alphaT[:, t * NE + o0:t * NE + o0 + 1])
                        for e in range(o0 + 1, GRP_OFF[d + 1]):
                            s = EDGES[e][0]
                            ge = cfg.agg_engines[gi % len(cfg.agg_engines)]
                            gi += 1
                            eng(ge).scalar_tensor_tensor(
                                out_t[:, d * C:(d + 1) * C],
                                xl_gm[:, s * C:(s + 1) * C],
                                alphaT[:, t * NE + e:t * NE + e + 1],
                                out_t[:, d * C:(d + 1) * C],
                                op0=mybir.AluOpType.mult,
                                op1=mybir.AluOpType.add)
                    nc.sync.dma_start(
                        out_d[b * G + t * 128:b * G + (t + 1) * 128, :],
                        out_t[:])

    nc.compile()
    return nc


def make_host_inputs(x, W_l, W_r, att, cfg: Cfg):
    """Builds the per-core input maps (host-side sharding + layout prep)."""
    x = np.asarray(x, dtype=np.float32)
    W_l = np.ascontiguousarray(np.asarray(W_l, dtype=np.float32))
    W_r = np.ascontiguousarray(np.asarray(W_r, dtype=np.float32))
    att = np.asarray(att, dtype=np.float32)
    bf = ml_dtypes.bfloat16

    # 0.8-scaled att (x) onehot(e) stationary blocks for the relu part
    attbl = np.zeros((128, 2, NE, NE), dtype=np.float32)
    for dch in range(2):
        for e in range(NE):
            attbl[:, dch, e, e] = 0.8 * att[dch * 128:(dch + 1) * 128]
    attbl = attbl.reshape(128, 2 * NE * NE).astype(bf)

    # 0.2-scaled [att@W_l | att@W_r] one-hot-block stationaries for nu
    wattl = 0.2 * (W_l @ att)     # [256]
    wattr = 0.2 * (W_r @ att)
    wattbl = np.zeros((128, 2, NN, 18), dtype=np.float32)
    for cc in range(2):
        for s in range(NN):
            wattbl[:, cc, s, 2 * s] = wattl[cc * 128:(cc + 1) * 128]
            wattbl[:, cc, s, 2 * s + 1] = wattr[cc * 128:(cc + 1) * 128]
    wattbl = wattbl.reshape(128, 2 * NN * 18).astype(bf)

    # selector: score_e += nu_l[src_e] + nu_r[dst_e]
    vsel = np.zeros((18, NE), dtype=np.float32)
    for e, (s, d) in enumerate(EDGES):
        vsel[2 * s, e] = 1.0
        vsel[2 * d + 1, e] = 1.0

    smat = np.zeros((NE, NE), dtype=np.float32)
    for e1, (_s1, d1) in enumerate(EDGES):
        for e2, (_s2, d2) in enumerate(EDGES):
            if d1 == d2:
                smat[e1, e2] = 1.0

    ident = np.eye(128, dtype=np.float32)

    in_maps = []
    for c in range(N_CORES):
        xc = x[c * BC:(c + 1) * BC]                       # [BC, 9, 256]
        xT = np.ascontiguousarray(
            xc.reshape(NBLK, G, NN, C).transpose(3, 0, 2, 1).reshape(
                C, NBLK * NGT).astype(bf))
        in_maps.append({
            "xT": xT,
            "wl": W_l.astype(bf),
            "wr": W_r.astype(bf),
            "attbl": attbl,
            "wattbl": wattbl,
            "vsel": vsel,
            "smat": smat,
            "identf": ident,
            "identb": ident.astype(bf),
        })
    return in_maps


_CACHE = {}


def _cfg_key(cfg: Cfg):
    return (cfg.add_engines, cfg.relu_engines, cfg.agg_engines,
            cfg.copy_engines, cfg.aggi_engines, cfg.proj_pair)


def _get_program(cfg: Cfg):
    key = _cfg_key(cfg)
    if key not in _CACHE:
        _CACHE[key] = build_program(cfg)
    return _CACHE[key]


def kernel(x, W_l, W_r, att, bias, cfg: Cfg = None, trace: bool = False,
           _results_holder: dict = None, **run_kwargs):
    cfg = cfg or Cfg()
    nc = _get_program(cfg)
    in_maps = make_host_inputs(x, W_l, W_r, att, cfg)
    res = run_bass_kernel_spmd(nc, in_maps, core_ids=list(range(N_CORES)),
                               trace=trace, **run_kwargs)
    if _results_holder is not None:
        _results_holder["res"] = res
    outs = [np.asarray(r["out"], dtype=np.float32).reshape(BC, NN, C)
            for r in res.results]
    out = np.concatenate(outs, axis=0)
    bias = np.asarray(bias, dtype=np.float32)
    if np.any(bias):
        out = out + bias
    return out.astype(np.float32)


# revision 22
# speedup vs baseline: 3.0341x; 1.1295x over previous
"""GATv2Conv batched-graph kernel for Trainium2 (8 NeuronCores, data-parallel).

Problem: B=16384 independent 9-node graphs, C_in=C_out=256, fixed edge list
(16 directed tree edges + 9 self-loops = 25 edges), GATv2 attention.

Per core (B/8 = 2048 graphs), per block of G=512 graphs:
  - host pre-transposes x to channel-major node-major layout
    xT[c, blk*9*G + s*G + g] (bf16) so all matmuls/vector slices are
    contiguous.
  - PE: channel-major xlT/xrT projections; graph-major xl_gm projection
    (lhsT = xT slices, rhs = W) for the aggregation; per-edge score
    matmuls att . leaky(s_e) accumulated into one [25, G] PSUM tile via
    att (x) onehot(e) stationaries; a subset of the per-edge adds as
    identity-stationary accumulating matmuls (s_e = I.T@xl_s + I.T@xr_d
    in PSUM); denominator segment-sum via a [25, 25] indicator matmul;
    small transposes for per-graph alpha.
  - DVE/GPSIMD: remaining per-edge adds (bf16 TT), leaky-relu as one
    fused scalar_tensor_tensor op max(0.2*s, s), and the alpha-weighted
    aggregation as non-RMW scalar_tensor_tensor chains in graph-major
    layout (alpha is a per-partition scalar there).
  - ACT: exp, PSUM->SBUF projection copies, some aggregation-chain heads
    (copy with scale).
  - Software pipelining: block b's projections are emitted interleaved
    with block b-1's softmax tail + aggregation so the PE never drains
    (keeps the HAM clock gate warm) and the vector engines always have
    ready work.
  - softmax needs no max-subtraction (scores are O(1)); bias is handled
    host-side (it is zeros in this problem).
"""

import sys

if "/opt/trn_rl_repo" not in sys.path:
    sys.path.insert(0, "/opt/trn_rl_repo")

import numpy as np
import ml_dtypes

import concourse.bass as bass
import concourse.bacc as bacc
import concourse.mybir as mybir
from concourse import tile
from concourse.bass_utils import run_bass_kernel_spmd

F32 = mybir.dt.float32
BF16 = mybir.dt.bfloat16

N_CORES = 8
B_TOTAL = 16384
BC = B_TOTAL // N_CORES          # graphs per core
NN = 9                           # nodes per graph
C = 256                          # channels
G = 512                          # graphs per block
NBLK = BC // G                   # blocks per core
NT = G // 128                    # 128-graph subtiles per block
NGT = NN * G                     # columns per (chunk, block)

# ---- static edge list, grouped by destination, self-loop first in group ----
_ADJ = {0: [1, 3, 5, 7], 1: [0, 2], 2: [1], 3: [0, 4], 4: [3],
        5: [0, 6], 6: [5], 7: [0, 8], 8: [7]}
EDGES = []          # (src, dst), sorted by dst, self-loop first
GRP_OFF = [0]       # group offsets into EDGES per dst
for _d in range(NN):
    EDGES.append((_d, _d))
    for _s in _ADJ[_d]:
        EDGES.append((_s, _d))
    GRP_OFF.append(len(EDGES))
NE = len(EDGES)     # 25
assert NE == 25


class Cfg:
    # per-edge-chunk adds (TT add: DVE 1cpe, Pool ~2.9cpe)
    add_engines = ("vector", "gpsimd")
    # relu for SBUF-sourced adds (baseline split; no Pool TS - 9us trap)
    relu_engines = ("vector", "scalar")
    # aggregation chain STT ops (DVE-only)
    stt_engines = ("vector",)
    # aggregation chain head (copy/mul with per-partition scale)
    first_engines = ("scalar", "scalar", "vector")
    # psum->sbuf projection copies
    copy_engines = ("scalar",)
    # xl_gm psum->sbuf copies
    gm_copy_engines = ("scalar",)
    proj_pair = 2                # 512-col groups per proj psum
    n_pe_adds = 0                # PE adds disabled (ring pressure hurt)


def build_program(cfg: Cfg):
    nc = bacc.Bacc("TRN2", target_bir_lowering=False, debug=False)

    def eng(name):
        return {"vector": nc.vector, "gpsimd": nc.gpsimd,
                "scalar": nc.scalar}[name]

    def copy_op(ename, dst_ap, src_ap):
        if ename == "scalar":
            nc.scalar.copy(dst_ap, src_ap)
        else:
            eng(ename).tensor_copy(dst_ap, src_ap)

    # DRAM tensors
    xT_d = nc.dram_tensor("xT", [C, NBLK * NGT], BF16, kind="ExternalInput")
    wl_d = nc.dram_tensor("wl", [C, C], BF16, kind="ExternalInput")
    wr_d = nc.dram_tensor("wr", [C, C], BF16, kind="ExternalInput")
    attbl_d = nc.dram_tensor("attbl", [128, 2 * NE * NE], BF16,
                             kind="ExternalInput")
    wattbl_d = nc.dram_tensor("wattbl", [128, 2 * NN * 18], BF16,
                              kind="ExternalInput")
    vsel_d = nc.dram_tensor("vsel", [18, NE], F32, kind="ExternalInput")
    smat_d = nc.dram_tensor("smat", [NE, NE], F32, kind="ExternalInput")
    identf_d = nc.dram_tensor("identf", [128, 128], F32, kind="ExternalInput")
    identb_d = nc.dram_tensor("identb", [128, 128], BF16, kind="ExternalInput")
    out_d = nc.dram_tensor("out", [BC, NN * C], BF16, kind="ExternalOutput")

    with tile.TileContext(nc) as tc:
        with (
            tc.tile_pool(name="const", bufs=1) as cpool,
            tc.tile_pool(name="xin", bufs=2) as xpool,
            tc.tile_pool(name="proj", bufs=1) as prpool,
            tc.tile_pool(name="edge", bufs=6) as epool,
            tc.tile_pool(name="soft", bufs=2) as spool,
            tc.tile_pool(name="gm", bufs=8) as gmpool,
            tc.tile_pool(name="chain", bufs=4) as chpool,
            tc.tile_pool(name="outp", bufs=4) as opool,
            tc.tile_pool(name="ps_proj", bufs=2, space="PSUM") as ps_proj,
            tc.tile_pool(name="ps_gm", bufs=2, space="PSUM") as ps_gm,
            tc.tile_pool(name="ps_sc", bufs=1, space="PSUM") as ps_sc,
            tc.tile_pool(name="ps_misc", bufs=1, space="PSUM") as ps_misc,
        ):
            # ---- constants ----
            wl_sb = cpool.tile([128, 2 * C], BF16, tag="wl")
            wr_sb = cpool.tile([128, 2 * C], BF16, tag="wr")
            nc.sync.dma_start(wl_sb[:, 0:C], wl_d[0:128, :])
            nc.sync.dma_start(wl_sb[:, C:2 * C], wl_d[128:256, :])
            nc.sync.dma_start(wr_sb[:, 0:C], wr_d[0:128, :])
            nc.sync.dma_start(wr_sb[:, C:2 * C], wr_d[128:256, :])
            attbl_sb = cpool.tile([128, 2 * NE * NE], BF16, tag="attbl")
            nc.sync.dma_start(attbl_sb[:], attbl_d[:])
            wattbl_sb = cpool.tile([128, 2 * NN * 18], BF16, tag="wattbl")
            nc.sync.dma_start(wattbl_sb[:], wattbl_d[:])
            vsel_sb = cpool.tile([18, NE], F32, tag="vsel")
            nc.sync.dma_start(vsel_sb[:], vsel_d[:])
            smat_sb = cpool.tile([NE, NE], F32, tag="smat")
            nc.sync.dma_start(smat_sb[:], smat_d[:])
            identf_sb = cpool.tile([128, 128], F32, tag="identf")
            nc.sync.dma_start(identf_sb[:], identf_d[:])
            identb_sb = cpool.tile([128, 128], BF16, tag="identb")
            nc.sync.dma_start(identb_sb[:], identb_d[:])
            zero_sb = cpool.tile([128, G], BF16, tag="zero")
            nc.gpsimd.memset(zero_sb[:], 0.0)

            ci = 0    # proj copy engine cycler
            gci = 0   # gm copy engine cycler
            st_state = {}   # per-block saved tiles

            def make_proj_ops(b, xt):
                """20 closures: each runs the matmuls for one psum group and
                the psum->sbuf copy. Fills xlT/xrT maps in st_state[b]."""
                xlT = {}
                xrT = {}
                st_state[b]["xlT"] = xlT
                st_state[b]["xrT"] = xrT
                ops = []
                for (wsb, dest, nm) in ((wl_sb, xlT, "l"), (wr_sb, xrT, "r")):
                    for dch in range(2):
                        s = 0
                        while s < NN:
                            npair = min(cfg.proj_pair, NN - s)

                            def op(wsb=wsb, dest=dest, nm=nm, dch=dch, s=s,
                                   npair=npair):
                                nonlocal ci
                                ps = ps_proj.tile([128, cfg.proj_pair * G],
                                                  F32, tag="ps_proj")
                                for j in range(npair):
                                    nc.tensor.matmul(
                                        ps[:, j * G:(j + 1) * G],
                                        wsb[:, dch * 128:dch * 128 + 128],
                                        xt[0][:, (s + j) * G:(s + j + 1) * G],
                                        start=True, stop=False)
                                    nc.tensor.matmul(
                                        ps[:, j * G:(j + 1) * G],
                                        wsb[:, C + dch * 128:C + dch * 128
                                            + 128],
                                        xt[1][:, (s + j) * G:(s + j + 1) * G],
                                        start=False, stop=True)
                                dst = prpool.tile(
                                    [128, npair * G], BF16,
                                    tag=f"p{nm}{dch}{s}")
                                for j in range(npair):
                                    dest[(dch, s + j)] = (dst, j * G)
                                copy_op(
                                    cfg.copy_engines[ci % len(cfg.copy_engines)],
                                    dst[:], ps[:, 0:npair * G])
                                ci += 1

                            ops.append(op)
                            s += npair
                return ops

            def make_tail_ops(bp):
                """Softmax tail + aggregation for block bp, as closures."""
                S = st_state[bp]
                ops = []

                def t_exp():
                    ex_sb = spool.tile([NE, G], F32, tag="ex")
                    S["ex_sb"] = ex_sb
                    nc.scalar.activation(ex_sb[:], S["sc_ps"][:],
                                         mybir.ActivationFunctionType.Exp)
                ops.append(t_exp)

                def t_den():
                    den_ps = ps_misc.tile([NE, G], F32, tag="ps_misc")
                    S["den_ps"] = den_ps
                    nc.tensor.matmul(den_ps[:], smat_sb[:], S["ex_sb"][:],
                                     start=True, stop=True)
                ops.append(t_den)

                def t_den_copy():
                    den_sb = spool.tile([NE, G], F32, tag="den")
                    S["den_sb"] = den_sb
                    nc.scalar.copy(den_sb[:], S["den_ps"][:])
                ops.append(t_den_copy)

                def t_transpose():
                    exT_ps = ps_misc.tile([128, 2 * NT * NE], F32,
                                          tag="ps_misc")
                    S["exT_ps"] = exT_ps
                    dT0 = NT * NE
                    for t in range(NT):
                        nc.tensor.transpose(
                            exT_ps[:, t * NE:(t + 1) * NE],
                            S["ex_sb"][:, t * 128:(t + 1) * 128],
                            identf_sb[0:NE, 0:NE])
                        nc.tensor.transpose(
                            exT_ps[:, dT0 + t * NE:dT0 + (t + 1) * NE],
                            S["den_sb"][:, t * 128:(t + 1) * 128],
                            identf_sb[0:NE, 0:NE])
                ops.append(t_transpose)

                def t_alpha():
                    dT0 = NT * NE
                    exT_ps = S["exT_ps"]
                    rdenT = spool.tile([128, NT * NE], F32, tag="rdenT")
                    nc.vector.reciprocal(rdenT[:], exT_ps[:, dT0:2 * dT0])
                    alphaT = spool.tile([128, NT * NE], F32, tag="alphaT")
                    nc.vector.tensor_tensor(
                        alphaT[:], exT_ps[:, 0:dT0], rdenT[:],
                        op=mybir.AluOpType.mult)
                    S["alphaT"] = alphaT
                ops.append(t_alpha)

                # aggregation chains, one closure per (t, d) group; the out
                # DMA for tile t goes right after its last group.
                gi = [0]

                def agg_group(t, d, out_t):
                    S_ = st_state[bp]
                    alphaT = S_["alphaT"]
                    xl_gm = S_["xl_gms"][t]
                    o0 = GRP_OFF[d]
                    k = GRP_OFF[d + 1] - o0
                    # chain head: m = xl_gm[src0] * alpha[e0]
                    prev = None
                    for j in range(k):
                        e = o0 + j
                        s = EDGES[e][0]
                        last = (j == k - 1)
                        if last:
                            dst_ap = out_t[:, d * C:(d + 1) * C]
                        else:
                            cht = chpool.tile([128, C], BF16,
                                              tag=f"ch{gi[0] % 4}",
                                              name="cht")
                            dst_ap = cht[:]
                        if j == 0:
                            ie = cfg.first_engines[gi[0] %
                                                   len(cfg.first_engines)]
                            if ie == "scalar":
                                nc.scalar.activation(
                                    dst_ap, xl_gm[:, s * C:(s + 1) * C],
                                    mybir.ActivationFunctionType.Copy,
                                    scale=alphaT[:, t * NE + e:
                                                 t * NE + e + 1])
                            else:
                                eng(ie).tensor_scalar_mul(
                                    dst_ap, xl_gm[:, s * C:(s + 1) * C],
                                    alphaT[:, t * NE + e:t * NE + e + 1])
                        else:
                            ge = cfg.stt_engines[gi[0] % len(cfg.stt_engines)]
                            eng(ge).scalar_tensor_tensor(
                                dst_ap,
                                xl_gm[:, s * C:(s + 1) * C],
                                alphaT[:, t * NE + e:t * NE + e + 1],
                                prev,
                                op0=mybir.AluOpType.mult,
                                op1=mybir.AluOpType.add)
                        gi[0] += 1
                        prev = dst_ap

                for t in range(NT):
                    def t_aggtile(t=t):
                        out_t = opool.tile([128, NN * C], BF16, tag="out_t")
                        for d in range(NN):
                            agg_group(t, d, out_t)
                        nc.sync.dma_start(
                            out_d[bp * G + t * 128:bp * G + (t + 1) * 128, :],
                            out_t[:])
                    ops.append(t_aggtile)
                return ops

            def emit_edge_phase(b):
                S = st_state[b]
                xt = S["xt"]
                xlT, xrT = S["xlT"], S["xrT"]

                # graph-major xl projection pieces, interleaved into the edge
                # loop so PE has ready work while score matmuls wait on
                # DVE-made inputs
                xl_gms = [gmpool.tile([128, NN * C], BF16, tag="xl_gm",
                                      name="xl_gm")
                          for _ in range(NT)]
                S["xl_gms"] = xl_gms

                def gm_piece(t, s, ce):
                    npair = min(2, NN - s)
                    ps = ps_gm.tile([128, 512], F32, tag="ps_gm")
                    for j in range(npair):
                        nc.tensor.matmul(
                            ps[:, j * C:(j + 1) * C],
                            xt[0][:, (s + j) * G + t * 128:
                                  (s + j) * G + (t + 1) * 128],
                            wl_sb[:, 0:C],
                            start=True, stop=False)
                        nc.tensor.matmul(
                            ps[:, j * C:(j + 1) * C],
                            xt[1][:, (s + j) * G + t * 128:
                                  (s + j) * G + (t + 1) * 128],
                            wl_sb[:, C:2 * C],
                            start=False, stop=True)
                    copy_op(ce, xl_gms[t][:, s * C:(s + npair) * C],
                            ps[:, 0:npair * C])

                gm_pieces = [(t, s) for t in range(NT)
                             for s in (0, 2, 4, 6, 8)]

                nonlocal gci
                # nu = [0.2*att@W_l | 0.2*att@W_r] . x  -> [18, G]
                nu_ps = ps_misc.tile([18, G], F32, tag="ps_misc")
                for s in range(NN):
                    for cc in range(2):
                        nc.tensor.matmul(
                            nu_ps[:],
                            wattbl_sb[:, (cc * NN + s) * 18:
                                      (cc * NN + s + 1) * 18],
                            xt[cc][:, s * G:(s + 1) * G],
                            start=(s == 0 and cc == 0),
                            stop=(s == NN - 1 and cc == 1))
                nu_sb = spool.tile([18, G], F32, tag="nu")
                nc.vector.tensor_copy(nu_sb[:], nu_ps[:])

                sc_ps = ps_sc.tile([NE, G], F32, tag="ps_sc")
                S["sc_ps"] = sc_ps
                ai = 0   # add engine cycler
                ri = 0   # relu engine cycler

                def relu_op(ename, dst_ap, src_ap):
                    if ename == "scalar":
                        nc.scalar.activation(
                            dst_ap, src_ap,
                            mybir.ActivationFunctionType.Relu)
                    elif ename == "vector":
                        nc.vector.tensor_scalar_max(dst_ap, src_ap, 0.0)
                    else:
                        nc.scalar.activation(
                            dst_ap, src_ap,
                            mybir.ActivationFunctionType.Relu)

                for e, (s, d) in enumerate(EDGES):
                    if e < len(gm_pieces):
                        gt, gs = gm_pieces[e]
                        gm_piece(gt, gs,
                                 cfg.gm_copy_engines[gci %
                                                     len(cfg.gm_copy_engines)])
                        gci += 1
                    for dch in range(2):
                        idx = e * 2 + dch
                        lt, lo = xlT[(dch, s)]
                        rt, ro = xrT[(dch, d)]
                        lk = epool.tile([128, G], BF16, tag="lk")
                        if idx >= 50 - cfg.n_pe_adds:
                            # PE add: s_e = I.T @ xl_s + I.T @ xr_d in psum;
                            # relu on ACT (reads PSUM)
                            ps = ps_gm.tile([128, 512], F32, tag="ps_pe")
                            nc.tensor.matmul(ps[:], identb_sb[:],
                                             lt[:, lo:lo + G],
                                             start=True, stop=False)
                            nc.tensor.matmul(ps[:], identb_sb[:],
                                             rt[:, ro:ro + G],
                                             start=False, stop=True)
                            nc.scalar.activation(
                                lk[:], ps[:],
                                mybir.ActivationFunctionType.Relu)
                        else:
                            st = epool.tile([128, G], BF16, tag="st")
                            ae = cfg.add_engines[ai % len(cfg.add_engines)]
                            ai += 1
                            eng(ae).tensor_tensor(
                                st[:], lt[:, lo:lo + G], rt[:, ro:ro + G],
                                op=mybir.AluOpType.add)
                            re = cfg.relu_engines[ri % len(cfg.relu_engines)]
                            ri += 1
                            relu_op(re, lk[:], st[:])
                        blk = (dch * NE + e) * NE
                        nc.tensor.matmul(
                            sc_ps[:], attbl_sb[:, blk:blk + NE], lk[:],
                            start=(e == 0 and dch == 0), stop=False)
                # linear 0.2*att*s part for all edges, joins last
                nc.tensor.matmul(sc_ps[:], vsel_sb[:], nu_sb[:],
                                 start=False, stop=True)

            # ---------------- main pipelined loop ----------------
            for b in range(NBLK + 1):
                proj_ops = []
                if b < NBLK:
                    st_state[b] = {}
                    xt = []
                    for cc in range(2):
                        t = xpool.tile([128, NGT], BF16, tag=f"xt{cc}")
                        nc.sync.dma_start(
                            t[:], xT_d[cc * 128:(cc + 1) * 128,
                                       b * NGT:(b + 1) * NGT])
                        xt.append(t)
                    st_state[b]["xt"] = xt
                    proj_ops = make_proj_ops(b, xt)
                tail_ops = make_tail_ops(b - 1) if b >= 1 else []

                # interleave: tail's cheap softmax steps early (between proj
                # pieces so PE never stalls on the exp->den->transpose chain),
                # agg tiles spread through the remaining proj pieces.
                sched = []
                pi, ti = 0, 0
                # softmax chain: exp, den, den_copy, transpose, alpha (5 ops)
                softmax_n = 5 if tail_ops else 0
                while pi < len(proj_ops) or ti < len(tail_ops):
                    if ti < softmax_n:
                        if pi < len(proj_ops):
                            sched.append(proj_ops[pi]); pi += 1
                        if pi < len(proj_ops):
                            sched.append(proj_ops[pi]); pi += 1
                        sched.append(tail_ops[ti]); ti += 1
                    elif ti < len(tail_ops):
                        if pi < len(proj_ops):
                            sched.append(proj_ops[pi]); pi += 1
                        if pi < len(proj_ops):
                            sched.append(proj_ops[pi]); pi += 1
                        sched.append(tail_ops[ti]); ti += 1
                    else:
                        sched.append(proj_ops[pi]); pi += 1
                for op in sched:
                    op()

                if b < NBLK:
                    emit_edge_phase(b)

    nc.compile()
    return nc


def make_host_inputs(x, W_l, W_r, att, cfg: Cfg):
    """Builds the per-core input maps (host-side sharding + layout prep)."""
    x = np.asarray(x, dtype=np.float32)
    W_l = np.ascontiguousarray(np.asarray(W_l, dtype=np.float32))
    W_r = np.ascontiguousarray(np.asarray(W_r, dtype=np.float32))
    att = np.asarray(att, dtype=np.float32)
    bf = ml_dtypes.bfloat16

    # 0.8-scaled att (x) onehot(e) stationary blocks for the relu part
    attbl = np.zeros((128, 2, NE, NE), dtype=np.float32)
    for dch in range(2):
        for e in range(NE):
            attbl[:, dch, e, e] = 0.8 * att[dch * 128:(dch + 1) * 128]
    attbl = attbl.reshape(128, 2 * NE * NE).astype(bf)

    # 0.2-scaled [att@W_l | att@W_r] one-hot-block stationaries for nu
    wattl = 0.2 * (W_l @ att)     # [256]
    wattr = 0.2 * (W_r @ att)
    wattbl = np.zeros((128, 2, NN, 18), dtype=np.float32)
    for cc in range(2):
        for s in range(NN):
            wattbl[:, cc, s, 2 * s] = wattl[cc * 128:(cc + 1) * 128]
            wattbl[:, cc, s, 2 * s + 1] = wattr[cc * 128:(cc + 1) * 128]
    wattbl = wattbl.reshape(128, 2 * NN * 18).astype(bf)

    # selector: score_e += nu_l[src_e] + nu_r[dst_e]
    vsel = np.zeros((18, NE), dtype=np.float32)
    for e, (s, d) in enumerate(EDGES):
        vsel[2 * s, e] = 1.0
        vsel[2 * d + 1, e] = 1.0

    smat = np.zeros((NE, NE), dtype=np.float32)
    for e1, (_s1, d1) in enumerate(EDGES):
        for e2, (_s2, d2) in enumerate(EDGES):
            if d1 == d2:
                smat[e1, e2] = 1.0

    ident = np.eye(128, dtype=np.float32)

    in_maps = []
    for c in range(N_CORES):
        xc = x[c * BC:(c + 1) * BC]                       # [BC, 9, 256]
        xT = np.ascontiguousarray(
            xc.reshape(NBLK, G, NN, C).transpose(3, 0, 2, 1).reshape(
                C, NBLK * NGT).astype(bf))
        in_maps.append({
            "xT": xT,
            "wl": W_l.astype(bf),
            "wr": W_r.astype(bf),
            "attbl": attbl,
            "wattbl": wattbl,
            "vsel": vsel,
            "smat": smat,
            "identf": ident,
            "identb": ident.astype(bf),
        })
    return in_maps


_CACHE = {}


def _cfg_key(cfg: Cfg):
    return (cfg.add_engines, cfg.relu_engines, cfg.stt_engines,
            cfg.first_engines, cfg.copy_engines, cfg.gm_copy_engines,
            cfg.proj_pair, cfg.n_pe_adds)


def _get_program(cfg: Cfg):
    key = _cfg_key(cfg)
    if key not in _CACHE:
        _CACHE[key] = build_program(cfg)
    return _CACHE[key]


def kernel(x, W_l, W_r, att, bias, cfg: Cfg = None, trace: bool = False,
           _results_holder: dict = None, **run_kwargs):
    cfg = cfg or Cfg()
    nc = _get_program(cfg)
    in_maps = make_host_inputs(x, W_l, W_r, att, cfg)
    res = run_bass_kernel_spmd(nc, in_maps, core_ids=list(range(N_CORES)),
                               trace=trace, **run_kwargs)
    if _results_holder is not None:
        _results_holder["res"] = res
    outs = [np.asarray(r["out"], dtype=np.float32).reshape(BC, NN, C)
            for r in res.results]
    out = np.concatenate(outs, axis=0)
    bias = np.asarray(bias, dtype=np.float32)
    if np.any(bias):
        out = out + bias
    return out.astype(np.float32)


# revision 23
# speedup vs baseline: 3.2762x; 1.0798x over previous
"""GATv2Conv batched-graph kernel for Trainium2 (8 NeuronCores, data-parallel).

Per core (B/8 = 2048 graphs), per block of G=512 graphs:
  - host pre-transposes x to channel-major node-major layout (bf16).
  - PE: channel-major xlT/xrT projections; graph-major xl_gm projection;
    per-edge score matmuls 0.8*att*relu(s_e) accumulated into one [25, G]
    PSUM tile; nu linear part via one-hot stationaries + vsel selector;
    denominator segment-sum via a [25, 25] indicator matmul; small
    transposes for per-graph alpha.
  - DVE/GPSIMD: per-edge adds (bf16), relu, and the alpha-weighted
    aggregation with fused scalar_tensor_tensor ops in graph-major layout.
  - ACT: exp + most PSUM->SBUF copies.
"""

import sys

if "/opt/trn_rl_repo" not in sys.path:
    sys.path.insert(0, "/opt/trn_rl_repo")

import numpy as np
import ml_dtypes

import concourse.bass as bass
import concourse.bacc as bacc
import concourse.mybir as mybir
from concourse import tile
from concourse.bass_utils import run_bass_kernel_spmd

F32 = mybir.dt.float32
BF16 = mybir.dt.bfloat16

N_CORES = 8
B_TOTAL = 16384
BC = B_TOTAL // N_CORES          # graphs per core
NN = 9                           # nodes per graph
C = 256                          # channels
G = 512                          # graphs per block
NBLK = BC // G                   # blocks per core
NT = G // 128                    # 128-graph subtiles per block
NGT = NN * G                     # columns per (chunk, block)

# ---- static edge list, grouped by destination, self-loop first in group ----
_ADJ = {0: [1, 3, 5, 7], 1: [0, 2], 2: [1], 3: [0, 4], 4: [3],
        5: [0, 6], 6: [5], 7: [0, 8], 8: [7]}
EDGES = []          # (src, dst), sorted by dst, self-loop first
GRP_OFF = [0]       # group offsets into EDGES per dst
for _d in range(NN):
    EDGES.append((_d, _d))
    for _s in _ADJ[_d]:
        EDGES.append((_s, _d))
    GRP_OFF.append(len(EDGES))
NE = len(EDGES)     # 25
assert NE == 25


class Cfg:
    add_engines = ("vector", "gpsimd")       # per-edge adds round robin
    relu_engines = ("vector", "scalar")      # per-edge relu round robin
    agg_engines = ("vector",)                # agg STT round robin (AP scalar
                                             # ops are DVE-only)
    copy_engines = ("scalar",)               # psum->sbuf copies (ACT gap-fills)
    aggi_engines = ("scalar", "scalar", "vector")  # agg init: copy with scale
    proj_pair = 2                            # 512-col groups per proj psum


def build_program(cfg: Cfg):
    nc = bacc.Bacc("TRN2", target_bir_lowering=False, debug=False)

    def eng(name):
        return {"vector": nc.vector, "gpsimd": nc.gpsimd,
                "scalar": nc.scalar}[name]

    def copy_op(ename, dst_ap, src_ap):
        if ename == "scalar":
            nc.scalar.copy(dst_ap, src_ap)
        else:
            eng(ename).tensor_copy(dst_ap, src_ap)

    # DRAM tensors
    xT_d = nc.dram_tensor("xT", [C, NBLK * NGT], BF16, kind="ExternalInput")
    wl_d = nc.dram_tensor("wl", [C, C], BF16, kind="ExternalInput")
    wr_d = nc.dram_tensor("wr", [C, C], BF16, kind="ExternalInput")
    attbl_d = nc.dram_tensor("attbl", [128, 2 * NE * NE], BF16,
                             kind="ExternalInput")
    wattbl_d = nc.dram_tensor("wattbl", [128, 2 * NN * 18], BF16,
                              kind="ExternalInput")
    vsel_d = nc.dram_tensor("vsel", [18, NE], F32, kind="ExternalInput")
    smat_d = nc.dram_tensor("smat", [NE, NE], F32, kind="ExternalInput")
    identf_d = nc.dram_tensor("identf", [128, 128], F32, kind="ExternalInput")
    out_d = nc.dram_tensor("out", [BC, NN * C], BF16, kind="ExternalOutput")

    with tile.TileContext(nc) as tc:
        with (
            tc.tile_pool(name="const", bufs=1) as cpool,
            tc.tile_pool(name="xin", bufs=2) as xpool,
            tc.tile_pool(name="proj", bufs=2) as prpool,
            tc.tile_pool(name="edge", bufs=10) as epool,
            tc.tile_pool(name="soft", bufs=3) as spool,
            tc.tile_pool(name="gm", bufs=6) as gmpool,
            tc.tile_pool(name="outp", bufs=4) as opool,
            tc.tile_pool(name="ps_proj", bufs=2, space="PSUM") as ps_proj,
            tc.tile_pool(name="ps_gm", bufs=2, space="PSUM") as ps_gm,
            tc.tile_pool(name="ps_sc", bufs=1, space="PSUM") as ps_sc,
            tc.tile_pool(name="ps_misc", bufs=1, space="PSUM") as ps_misc,
        ):
            # ---- constants ----
            wl_sb = cpool.tile([128, 2 * C], BF16, tag="wl")
            wr_sb = cpool.tile([128, 2 * C], BF16, tag="wr")
            nc.sync.dma_start(wl_sb[:, 0:C], wl_d[0:128, :])
            nc.sync.dma_start(wl_sb[:, C:2 * C], wl_d[128:256, :])
            nc.sync.dma_start(wr_sb[:, 0:C], wr_d[0:128, :])
            nc.sync.dma_start(wr_sb[:, C:2 * C], wr_d[128:256, :])
            attbl_sb = cpool.tile([128, 2 * NE * NE], BF16, tag="attbl")
            nc.sync.dma_start(attbl_sb[:], attbl_d[:])
            wattbl_sb = cpool.tile([128, 2 * NN * 18], BF16, tag="wattbl")
            nc.sync.dma_start(wattbl_sb[:], wattbl_d[:])
            vsel_sb = cpool.tile([18, NE], F32, tag="vsel")
            nc.sync.dma_start(vsel_sb[:], vsel_d[:])
            smat_sb = cpool.tile([NE, NE], F32, tag="smat")
            nc.sync.dma_start(smat_sb[:], smat_d[:])
            identf_sb = cpool.tile([128, 128], F32, tag="identf")
            nc.sync.dma_start(identf_sb[:], identf_d[:])
            zero_sb = cpool.tile([128, G], BF16, tag="zero")
            nc.gpsimd.memset(zero_sb[:], 0.0)

            ci = 0   # copy engine cycler
            for b in range(NBLK):
                # ---- load xT block: 2 channel-chunks [128, NN*G] ----
                xt = []
                for cc in range(2):
                    t = xpool.tile([128, NGT], BF16, tag=f"xt{cc}")
                    nc.sync.dma_start(
                        t[:], xT_d[cc * 128:(cc + 1) * 128,
                                   b * NGT:(b + 1) * NGT])
                    xt.append(t)

                # ---- channel-major projections, one tile per psum copy ----
                # fine-grained tiles so edge adds gate on single copies
                xlT = {}   # (dch, s) -> (tile, col offset)
                xrT = {}
                for (wsb, dest, nm) in ((wl_sb, xlT, "l"), (wr_sb, xrT, "r")):
                    for dch in range(2):
                        s = 0
                        while s < NN:
                            npair = min(cfg.proj_pair, NN - s)
                            ps = ps_proj.tile([128, cfg.proj_pair * G], F32,
                                              tag="ps_proj")
                            for j in range(npair):
                                nc.tensor.matmul(
                                    ps[:, j * G:(j + 1) * G],
                                    wsb[:, dch * 128:dch * 128 + 128],
                                    xt[0][:, (s + j) * G:(s + j + 1) * G],
                                    start=True, stop=False)
                                nc.tensor.matmul(
                                    ps[:, j * G:(j + 1) * G],
                                    wsb[:, C + dch * 128:C + dch * 128 + 128],
                                    xt[1][:, (s + j) * G:(s + j + 1) * G],
                                    start=False, stop=True)
                            dst = prpool.tile(
                                [128, npair * G], BF16,
                                tag=f"p{nm}{dch}{s}")
                            for j in range(npair):
                                dest[(dch, s + j)] = (dst, j * G)
                            copy_op(cfg.copy_engines[ci % len(cfg.copy_engines)],
                                    dst[:], ps[:, 0:npair * G])
                            ci += 1
                            s += npair

                # ---- graph-major xl for aggregation (stationary = xT) ----
                # emitted as closures interleaved into the edge loop so PE
                # has ready work while score matmuls wait on DVE-made inputs
                xl_gms = [gmpool.tile([128, NN * C], BF16, tag="xl_gm",
                                      name="xl_gm")
                          for _ in range(NT)]

                def gm_piece(t, s, ce):
                    npair = min(2, NN - s)
                    ps = ps_gm.tile([128, 512], F32, tag="ps_gm")
                    for j in range(npair):
                        nc.tensor.matmul(
                            ps[:, j * C:(j + 1) * C],
                            xt[0][:, (s + j) * G + t * 128:
                                  (s + j) * G + (t + 1) * 128],
                            wl_sb[:, 0:C],
                            start=True, stop=False)
                        nc.tensor.matmul(
                            ps[:, j * C:(j + 1) * C],
                            xt[1][:, (s + j) * G + t * 128:
                                  (s + j) * G + (t + 1) * 128],
                            wl_sb[:, C:2 * C],
                            start=False, stop=True)
                    copy_op(ce, xl_gms[t][:, s * C:(s + npair) * C],
                            ps[:, 0:npair * C])

                gm_pieces = [(t, s) for t in range(NT)
                             for s in (0, 2, 4, 6, 8)]

                # ---- nu = [0.2*att@W_l | 0.2*att@W_r] . x per node ----
                nu_ps = ps_misc.tile([18, G], F32, tag="ps_misc")
                for s in range(NN):
                    for cc in range(2):
                        nc.tensor.matmul(
                            nu_ps[:],
                            wattbl_sb[:, (cc * NN + s) * 18:
                                      (cc * NN + s + 1) * 18],
                            xt[cc][:, s * G:(s + 1) * G],
                            start=(s == 0 and cc == 0),
                            stop=(s == NN - 1 and cc == 1))
                nu_sb = spool.tile([18, G], F32, tag="nu")
                nc.vector.tensor_copy(nu_sb[:], nu_ps[:])

                # ---- edge phase: adds + relu + score matmuls ----
                sc_ps = ps_sc.tile([NE, G], F32, tag="ps_sc")
                for e, (s, d) in enumerate(EDGES):
                    if e < len(gm_pieces):
                        gt, gs = gm_pieces[e]
                        gm_piece(gt, gs,
                                 cfg.copy_engines[ci % len(cfg.copy_engines)])
                        ci += 1
                    for dch in range(2):
                        idx = e * 2 + dch
                        st = epool.tile([128, G], BF16, tag="st")
                        ae = cfg.add_engines[idx % len(cfg.add_engines)]
                        lt, lo = xlT[(dch, s)]
                        rt, ro = xrT[(dch, d)]
                        eng(ae).tensor_tensor(
                            st[:], lt[:, lo:lo + G], rt[:, ro:ro + G],
                            op=mybir.AluOpType.add)
                        lk = epool.tile([128, G], BF16, tag="lk")
                        re = cfg.relu_engines[idx % len(cfg.relu_engines)]
                        if re == "scalar":
                            nc.scalar.activation(
                                lk[:], st[:],
                                mybir.ActivationFunctionType.Relu)
                        else:
                            eng(re).tensor_scalar_max(lk[:], st[:], 0.0)
                        blk = (dch * NE + e) * NE
                        nc.tensor.matmul(
                            sc_ps[:], attbl_sb[:, blk:blk + NE], lk[:],
                            start=(e == 0 and dch == 0), stop=False)
                # linear 0.2*att*s part for all edges, joins last
                nc.tensor.matmul(sc_ps[:], vsel_sb[:], nu_sb[:],
                                 start=False, stop=True)

                # ---- softmax (no max-subtraction; scores are O(1)) ----
                ex_sb = spool.tile([NE, G], F32, tag="ex")
                nc.scalar.activation(ex_sb[:], sc_ps[:],
                                     mybir.ActivationFunctionType.Exp)
                # per-edge gathered denominator: M[e',e] = [dst match]
                den_ps = ps_misc.tile([NE, G], F32, tag="ps_misc")
                nc.tensor.matmul(den_ps[:], smat_sb[:], ex_sb[:],
                                 start=True, stop=True)
                den_sb = spool.tile([NE, G], F32, tag="den")
                nc.scalar.copy(den_sb[:], den_ps[:])

                # transposed per-128-graph alpha: one TT per block
                exT_ps = ps_misc.tile([128, 2 * NT * NE], F32,
                                      tag="ps_misc")
                dT0 = NT * NE
                for t in range(NT):
                    nc.tensor.transpose(
                        exT_ps[:, t * NE:(t + 1) * NE],
                        ex_sb[:, t * 128:(t + 1) * 128],
                        identf_sb[0:NE, 0:NE])
                    nc.tensor.transpose(
                        exT_ps[:, dT0 + t * NE:dT0 + (t + 1) * NE],
                        den_sb[:, t * 128:(t + 1) * 128],
                        identf_sb[0:NE, 0:NE])
                rdenT = spool.tile([128, NT * NE], F32, tag="rdenT")
                nc.vector.reciprocal(rdenT[:], exT_ps[:, dT0:dT0 + 2 * dT0 - dT0])
                alphaT = spool.tile([128, NT * NE], F32, tag="alphaT")
                nc.vector.tensor_tensor(
                    alphaT[:], exT_ps[:, 0:dT0], rdenT[:],
                    op=mybir.AluOpType.mult)

                # ---- aggregation in graph-major ----
                gi = 0
                for t in range(NT):
                    xl_gm = xl_gms[t]
                    out_t = opool.tile([128, NN * C], BF16, tag="out_t")
                    for d in range(NN):
                        o0 = GRP_OFF[d]
                        ie = cfg.aggi_engines[gi % len(cfg.aggi_engines)]
                        if ie == "scalar":
                            nc.scalar.activation(
                                out_t[:, d * C:(d + 1) * C],
                                xl_gm[:, d * C:(d + 1) * C],
                                mybir.ActivationFunctionType.Copy,
                                scale=alphaT[:, t * NE + o0:t * NE + o0 + 1])
                        else:
                            eng(ie).tensor_scalar_mul(
                                out_t[:, d * C:(d + 1) * C],
                                xl_gm[:, d * C:(d + 1) * C],
                                alphaT[:, t * NE + o0:t * NE + o0 + 1])
                        for e in range(o0 + 1, GRP_OFF[d + 1]):
                            s = EDGES[e][0]
                            ge = cfg.agg_engines[gi % len(cfg.agg_engines)]
                            gi += 1
                            eng(ge).scalar_tensor_tensor(
                                out_t[:, d * C:(d + 1) * C],
                                xl_gm[:, s * C:(s + 1) * C],
                                alphaT[:, t * NE + e:t * NE + e + 1],
                                out_t[:, d * C:(d + 1) * C],
                                op0=mybir.AluOpType.mult,
                                op1=mybir.AluOpType.add)
                    nc.sync.dma_start(
                        out_d[b * G + t * 128:b * G + (t + 1) * 128, :],
                        out_t[:])

    nc.compile()
    return nc


def make_host_inputs(x, W_l, W_r, att, cfg: Cfg):
    """Builds the per-core input maps (host-side sharding + layout prep)."""
    x = np.asarray(x, dtype=np.float32)
    W_l = np.ascontiguousarray(np.asarray(W_l, dtype=np.float32))
    W_r = np.ascontiguousarray(np.asarray(W_r, dtype=np.float32))
    att = np.asarray(att, dtype=np.float32)
    bf = ml_dtypes.bfloat16

    # 0.8-scaled att (x) onehot(e) stationary blocks for the relu part
    attbl = np.zeros((128, 2, NE, NE), dtype=np.float32)
    for dch in range(2):
        for e in range(NE):
            attbl[:, dch, e, e] = 0.8 * att[dch * 128:(dch + 1) * 128]
    attbl = attbl.reshape(128, 2 * NE * NE).astype(bf)

    # 0.2-scaled [att@W_l | att@W_r] one-hot-block stationaries for nu
    wattl = 0.2 * (W_l @ att)     # [256]
    wattr = 0.2 * (W_r @ att)
    wattbl = np.zeros((128, 2, NN, 18), dtype=np.float32)
    for cc in range(2):
        for s in range(NN):
            wattbl[:, cc, s, 2 * s] = wattl[cc * 128:(cc + 1) * 128]
            wattbl[:, cc, s, 2 * s + 1] = wattr[cc * 128:(cc + 1) * 128]
    wattbl = wattbl.reshape(128, 2 * NN * 18).astype(bf)

    # selector: score_e += nu_l[src_e] + nu_r[dst_e]
    vsel = np.zeros((18, NE), dtype=np.float32)
    for e, (s, d) in enumerate(EDGES):
        vsel[2 * s, e] = 1.0
        vsel[2 * d + 1, e] = 1.0

    smat = np.zeros((NE, NE), dtype=np.float32)
    for e1, (_s1, d1) in enumerate(EDGES):
        for e2, (_s2, d2) in enumerate(EDGES):
            if d1 == d2:
                smat[e1, e2] = 1.0

    ident = np.eye(128, dtype=np.float32)

    in_maps = []
    for c in range(N_CORES):
        xc = x[c * BC:(c + 1) * BC]                       # [BC, 9, 256]
        xT = np.ascontiguousarray(
            xc.reshape(NBLK, G, NN, C).transpose(3, 0, 2, 1).reshape(
                C, NBLK * NGT).astype(bf))
        in_maps.append({
            "xT": xT,
            "wl": W_l.astype(bf),
            "wr": W_r.astype(bf),
            "attbl": attbl,
            "wattbl": wattbl,
            "vsel": vsel,
            "smat": smat,
            "identf": ident,
        })
    return in_maps


_CACHE = {}


def _cfg_key(cfg: Cfg):
    return (cfg.add_engines, cfg.relu_engines, cfg.agg_engines,
            cfg.copy_engines, cfg.aggi_engines, cfg.proj_pair)


def _get_program(cfg: Cfg):
    key = _cfg_key(cfg)
    if key not in _CACHE:
        _CACHE[key] = build_program(cfg)
    return _CACHE[key]


def kernel(x, W_l, W_r, att, bias, cfg: Cfg = None, trace: bool = False,
           _results_holder: dict = None, **run_kwargs):
    cfg = cfg or Cfg()
    nc = _get_program(cfg)
    in_maps = make_host_inputs(x, W_l, W_r, att, cfg)
    res = run_bass_kernel_spmd(nc, in_maps, core_ids=list(range(N_CORES)),
                               trace=trace, **run_kwargs)
    if _results_holder is not None:
        _results_holder["res"] = res
    outs = [np.asarray(r["out"], dtype=np.float32).reshape(BC, NN, C)
            for r in res.results]
    out = np.concatenate(outs, axis=0)
    bias = np.asarray(bias, dtype=np.float32)
    if np.any(bias):
        out = out + bias
    return out.astype(np.float32)
